# revision 1
# baseline (speedup 1.0000x reference)
"""AFNO2D layer on 8 TRN2 NeuronCores.

Sharding: channel-block parallel. Core i owns channels [96*i, 96*(i+1)) —
exactly block i of the block-diagonal MLP. No collectives.

Per core, per batch sample (tokens t = hk*65 + wc, NT = 8320):
  S1  H-DFT  (data-stationary per channel pair, one [128,512] psum / 2 ch)
      lhsT=x_c [h,w], rhs=fh=[Ch|Sh] -> Zt [w, hk r|i, c]
  S2  W-rDFT (data-stationary per hk, 3 hk per [96,390] psum)
      lhsT=Zt[:,hk,:] / Zt[:,128+hk,:], rhs=fw -> Xri ring [c+1, hk, wc r|i]
  L1  MLP layer 1, bias via ones-row in Xri + bias-row in W1P; relu drain
  L2  MLP layer 2, bias in matmul; softshrink drain = v - clip(v)
  P1  pivot c->hk partitions: 130 DMA transposes [96,128] -> Y2 ring [hk,...]
  IH  H-iDFT F-stationary: lhsT=fhi3 [hk,h] -> Z [h, ri, wc(pad 128), c]
      (Z shares the Zt allocation; S1 fully rewrites it each sample)
  P2  pivot h->wc partitions (c-halves): DMA transposes -> Zp [wc.., ri, 48, h]
  IW  W-irDFT F-stationary: lhsT=fwi2=[Cwi|-Swi] -> [w, 512]-chunks -> HBM
Residual add + final transpose run on the host in fp32.
"""
import sys
import types
import numpy as np
import ml_dtypes

# run_bass_kernel_spmd(trace=True) needs this hook module; missing in image.
if "antenv.axon_hooks" not in sys.modules:
    _hooks_mod = types.ModuleType("antenv.axon_hooks")
    _hooks_mod._hook = None
    _hooks_mod.set_axon_ntff_profile_hook = lambda h: setattr(_hooks_mod, "_hook", h)
    _hooks_mod.get_axon_ntff_profile_hook = lambda: _hooks_mod._hook
    sys.modules["antenv.axon_hooks"] = _hooks_mod
    try:
        sys.path.insert(0, "/root/.axon_site")
        from trn_agent_boot.trn_boot import _ntff_profile_via_ctypes
        _hooks_mod._hook = _ntff_profile_via_ctypes("/opt/axon/libaxon_pjrt.so")
    except Exception:
        pass

import concourse.bacc as bacc
import concourse.tile as tile
from concourse import mybir
from concourse.bass_utils import run_bass_kernel_spmd

F32 = mybir.dt.float32
BF16 = mybir.dt.bfloat16

B, H, W, C = 4, 128, 128, 768
Wc = W // 2 + 1            # 65
NCORES, BLK = 8, 96        # channels per core
NT = H * Wc                # 8320 tokens per sample
LAM = 0.01
CHK = 455                  # 7 hk per MLP chunk
NCH = 19                   # MLP chunks (18*7 + 2 hk)

_cache = {}


def _build_consts():
    bf = ml_dtypes.bfloat16
    h = np.arange(H)
    hk = np.arange(H)
    wc = np.arange(Wc)
    w = np.arange(W)
    ang_h = 2 * np.pi * np.outer(h, hk) / H
    Ch, Sh = np.cos(ang_h) / np.sqrt(H), -np.sin(ang_h) / np.sqrt(H)
    ang_w = 2 * np.pi * np.outer(w, wc) / W
    Cw, Sw = np.cos(ang_w) / np.sqrt(W), -np.sin(ang_w) / np.sqrt(W)
    alpha = np.ones(Wc)
    alpha[1:64] = 2.0
    ang_wi = 2 * np.pi * np.outer(wc, w) / W
    Cwi = alpha[:, None] * np.cos(ang_wi) / np.sqrt(W)
    Swi = alpha[:, None] * np.sin(ang_wi) / np.sqrt(W)
    ang_hi = 2 * np.pi * np.outer(hk, h) / H
    Chi, Shi = np.cos(ang_hi) / np.sqrt(H), np.sin(ang_hi) / np.sqrt(H)

    fh = np.concatenate([Ch, Sh], axis=1).astype(bf)                   # [128,256]
    # one accumulation group: ztr @ [Cw|Sw] + zti @ [-Sw|Cw]
    fw = np.concatenate([Cw, Sw, -Sw, Cw], axis=1).astype(bf)          # [128,260]
    # iW (final, real out): out = Cwi^T Zpr + (-Swi)^T Zpi
    fwi2 = np.concatenate([Cwi, -Swi], axis=1).astype(bf)              # [65,256]
    # iH (complex): Zr = Chi^T Yr - Shi^T Yi ; Zi = Shi^T Yr + Chi^T Yi
    fhi3 = np.concatenate([Chi, -Shi, Shi], axis=1).astype(bf)         # [128,384]
    ones = np.ones((1, 2 * NT), dtype=np.float32).astype(bf)           # [1,16640]
    return fh, fw, fwi2, fhi3, ones


def _pack_mlp(w1, b1, w2, b2, blk):
    """[97, 384] packs: cols [Wr;br | -Wi;0 | Wi;bi | Wr;0]."""
    def pack(wr, wi, br, bi):
        p = np.zeros((97, 384), dtype=np.float32)
        p[:96, 0:96] = wr
        p[96, 0:96] = br
        p[:96, 96:192] = -wi
        p[:96, 192:288] = wi
        p[96, 192:288] = bi
        p[:96, 288:384] = wr
        return p
    w1p = pack(w1[0, blk], w1[1, blk], b1[0, blk], b1[1, blk])
    w2p = pack(w2[0, blk], w2[1, blk], b2[0, blk], b2[1, blk])
    return w1p, w2p


def _build_graph():
    nc = bacc.Bacc("TRN2", target_bir_lowering=False, debug=False,
                   num_devices=NCORES)

    x_ext = nc.dram_tensor("x", [B, H, W, BLK], F32, kind="ExternalInput").ap()
    fh_ext = nc.dram_tensor("fh", [128, 256], BF16, kind="ExternalInput").ap()
    fw_ext = nc.dram_tensor("fw", [128, 260], BF16, kind="ExternalInput").ap()
    fwi_ext = nc.dram_tensor("fwi2", [65, 256], BF16, kind="ExternalInput").ap()
    fhi_ext = nc.dram_tensor("fhi3", [128, 384], BF16, kind="ExternalInput").ap()
    w1_ext = nc.dram_tensor("w1p", [97, 384], F32, kind="ExternalInput").ap()
    w2_ext = nc.dram_tensor("w2p", [97, 384], F32, kind="ExternalInput").ap()
    on_ext = nc.dram_tensor("ones", [1, 2 * NT], BF16, kind="ExternalInput").ap()
    ml_ext = nc.dram_tensor("mlam", [96, 1], F32, kind="ExternalInput").ap()
    # device out: 24 chunks of [w, 512] over (c h)-flat; host reassembles
    out_ext = nc.dram_tensor("out", [B, 24, W, 512], BF16,
                             kind="ExternalOutput").ap()

    SUB = mybir.AluOpType.subtract
    MIN = mybir.AluOpType.min
    MAX = mybir.AluOpType.max
    RELU = mybir.ActivationFunctionType.Relu

    with tile.TileContext(nc) as tc:
        with (
            tc.tile_pool(name="consts", bufs=1) as cpool,
            tc.tile_pool(name="stat", bufs=1) as spool,
            tc.tile_pool(name="stg", bufs=3) as stg,      # clip staging
            tc.tile_pool(name="stg2", bufs=3) as stg2,    # out staging
            tc.tile_pool(name="psA", bufs=4, space="PSUM") as psA,  # [128,512]
            tc.tile_pool(name="psB", bufs=4, space="PSUM") as psB,  # [96,512]
        ):
            # ---- constants / weights to SBUF (once) ----
            fh = cpool.tile([128, 256], BF16, tag="fh")
            nc.sync.dma_start(out=fh, in_=fh_ext)
            fw = cpool.tile([128, 260], BF16, tag="fw")
            nc.sync.dma_start(out=fw, in_=fw_ext)
            fwi2 = cpool.tile([65, 256], BF16, tag="fwi2")
            nc.sync.dma_start(out=fwi2, in_=fwi_ext)
            fhi3 = cpool.tile([128, 384], BF16, tag="fhi3")
            nc.sync.dma_start(out=fhi3, in_=fhi_ext)
            w1p = cpool.tile([97, 384], BF16, tag="w1p")
            nc.gpsimd.dma_start(out=w1p, in_=w1_ext)      # casting DMA
            w2p = cpool.tile([97, 384], BF16, tag="w2p")
            nc.gpsimd.dma_start(out=w2p, in_=w2_ext)
            mlam = cpool.tile([96, 1], F32, tag="mlam")
            nc.sync.dma_start(out=mlam, in_=ml_ext)

            # W slices: lhsT [K, 96]
            W1ra = w1p[:, 0:96]          # [97, 96] row96 = b1r
            W1mi = w1p[0:96, 96:192]     # -Wi
            W1ib = w1p[:, 192:288]       # Wi ; b1i
            W1rb = w1p[0:96, 288:384]    # Wr
            W2ra = w2p[:, 0:96]
            W2mi = w2p[0:96, 96:192]
            W2ib = w2p[:, 192:288]
            W2rb = w2p[0:96, 288:384]

            # ---- static tiles (time-shared across stages/samples) ----
            X32f = spool.tile([128, W * BLK], BF16, tag="x32")
            X32 = X32f.rearrange("p (w c) -> p w c", c=BLK)
            ZZ = spool.tile([128, 24576], BF16, tag="zz")
            Zt = ZZ.rearrange("p (c a) -> p c a", c=BLK)       # [128,96,256]
            # iH out view [h, ri, c, wcpad]: wc contiguous for P2 DMA transpose
            Z = ZZ.rearrange("p (r c a) -> p r c a", r=2, c=BLK)  # [128,2,96,128]
            Xri = spool.tile([97, 2, 21, 130], BF16, tag="xri")   # 2-seg ring
            nc.sync.dma_start(out=Xri[96:97, :, :, :],
                              in_=on_ext[:, 0:2 * 21 * 130])
            O1 = spool.tile([97, 2, 3, CHK], BF16, tag="o1")      # 3-chunk ring
            nc.sync.dma_start(out=O1[96:97, :, :, :],
                              in_=on_ext[:, 0:2 * 3 * CHK])
            # wc-major so P1's DMA-transpose input [96, hk] is contiguous
            O2 = spool.tile([96, 2, Wc, H], BF16, tag="o2")
            Y2 = spool.tile([128, 2, Wc, BLK], BF16, tag="y2")
            Zp = spool.tile([128, 2, 2, 24, 128], BF16, tag="zp")  # 2 c-quarters

            for b in range(B):
                # ---- load sample (bufs=1: overwrites after S1(b-1) read) ----
                nc.gpsimd.dma_start(out=X32f, in_=x_ext[b])

                # alternate PSUM-drain engines (Pool cannot read PSUM)
                rr = [0]

                def drain(out, in_):
                    rr[0] += 1
                    if rr[0] % 2:
                        nc.vector.tensor_copy(out, in_)
                    else:
                        nc.scalar.copy(out, in_)

                # ---- S1: per channel pair -> Zt [w, c, hkri(256)] ----
                # both drain engines per psum: halves the drain latency
                for c0 in range(0, BLK, 2):
                    p1 = psA.tile([128, 512], F32, tag="psA")
                    nc.tensor.matmul(p1[:, 0:256], X32[:, :, c0], fh[:],
                                     start=True, stop=True)
                    nc.tensor.matmul(p1[:, 256:512], X32[:, :, c0 + 1], fh[:],
                                     start=True, stop=True)
                    drain(Zt[:, c0:c0 + 2, :], p1[:])

                # ---- S2 (3 hk / psum) then L1+L2 (7 hk / chunk), in 21-hk ----
                # ---- blocks so the 2-segment Xri ring stays coherent ----
                kchunk = 0
                for blk0 in range(0, H, 21):
                    bn = min(21, H - blk0)
                    seg = (blk0 // 21) % 2
                    for g0 in range(blk0, blk0 + bn, 3):
                        gn = min(3, blk0 + bn - g0)
                        off = g0 - blk0
                        p2 = psB.tile([96, 390], F32, tag="psB")
                        for j in range(gn):
                            hk = g0 + j
                            sl = slice(j * 130, (j + 1) * 130)
                            nc.tensor.matmul(p2[:, sl], Zt[:, :, hk],
                                             fw[:, 0:130], start=True, stop=False)
                            nc.tensor.matmul(p2[:, sl], Zt[:, :, 128 + hk],
                                             fw[:, 130:260], start=False,
                                             stop=True)
                        drain(Xri[0:96, seg, off:off + gn, :], p2[:, 0:gn * 130])

                    for h0 in range(blk0, blk0 + bn, 7):
                        hn = min(7, blk0 + bn - h0)
                        n = hn * Wc
                        off = h0 - blk0
                        xr = Xri[:, seg, off:off + hn, 0:65]
                        xi = Xri[:, seg, off:off + hn, 65:130]
                        pr = psB.tile([96, 512], F32, tag="psB")
                        pi = psB.tile([96, 512], F32, tag="psB")
                        nc.tensor.matmul(pr[:, :n], W1ra, xr, start=True,
                                         stop=False)
                        nc.tensor.matmul(pr[:, :n], W1mi, xi[0:96], start=False,
                                         stop=True)
                        nc.tensor.matmul(pi[:, :n], W1ib, xr, start=True,
                                         stop=False)
                        nc.tensor.matmul(pi[:, :n], W1rb, xi[0:96], start=False,
                                         stop=True)
                        kr = kchunk % 3
                        kchunk += 1
                        nc.vector.tensor_scalar(O1[0:96, 0, kr, :n], pr[:, :n],
                                                0.0, None, MAX)
                        nc.scalar.activation(O1[0:96, 1, kr, :n], pi[:, :n],
                                             RELU)

                        # L2 on the chunk just produced
                        ts = slice(h0 * Wc, h0 * Wc + n)
                        o1r = O1[:, 0, kr, :n]
                        o1i = O1[:, 1, kr, :n]
                        qr = psB.tile([96, 512], F32, tag="psB")
                        qi = psB.tile([96, 512], F32, tag="psB")
                        nc.tensor.matmul(qr[:, :n], W2ra, o1r, start=True,
                                         stop=False)
                        nc.tensor.matmul(qr[:, :n], W2mi, o1i[0:96], start=False,
                                         stop=True)
                        nc.tensor.matmul(qi[:, :n], W2ib, o1r, start=True,
                                         stop=False)
                        nc.tensor.matmul(qi[:, :n], W2rb, o1i[0:96], start=False,
                                         stop=True)
                        # O2 dst written contiguously (wc-major); psum operands
                        # read with (wc, hk)-permuted APs instead
                        o2r = O2[:, 0, :, h0:h0 + hn]
                        o2i = O2[:, 1, :, h0:h0 + hn]
                        qrv = qr[:, :n].rearrange("p (a b) -> p b a", b=Wc)
                        qiv = qi[:, :n].rearrange("p (a b) -> p b a", b=Wc)
                        # real: softshrink = y - clip(y) on DVE
                        t1 = stg.tile([96, CHK], F32, tag="clip")
                        t1v = t1[:, :n].rearrange("p (a b) -> p b a", b=Wc)
                        nc.vector.tensor_scalar(t1[:, :n], qr[:, :n], LAM, -LAM,
                                                MIN, MAX)
                        nc.vector.tensor_tensor(o2r, qrv, t1v, SUB)
                        # imag: relu(y-lam) - relu(-y-lam) on Act, sub on Pool
                        sa = stg.tile([96, CHK], BF16, tag="sa")
                        sb = stg.tile([96, CHK], BF16, tag="sb")
                        sav = sa[:, :n].rearrange("p (a b) -> p b a", b=Wc)
                        sbv = sb[:, :n].rearrange("p (a b) -> p b a", b=Wc)
                        nc.scalar.activation(sa[:, :n], qi[:, :n], RELU,
                                             bias=mlam)
                        nc.scalar.activation(sb[:, :n], qi[:, :n], RELU,
                                             bias=mlam, scale=-1.0)
                        nc.gpsimd.tensor_tensor(o2i, sav, sbv, SUB)

                # ---- P1: one batched DMA transpose per r/i ----
                # in [96, (wc.128hk)] -> out [128hk, wc, 96c]
                O2f = O2.rearrange("p r a b -> p r (a b)")
                nc.sync.dma_start(out=Y2[:, 0], in_=O2f[:, 0], transpose=True)
                nc.sync.dma_start(out=Y2[:, 1], in_=O2f[:, 1], transpose=True)

                # ---- IH: chunks of 7 channels (c-group) -> Z [h,ri,c,wcpad] ----
                for c0 in range(0, BLK, 7):
                    cn = min(7, BLK - c0)
                    n = cn * Wc
                    # moving cols ordered (c, wc): wc fastest
                    yr = Y2[:, 0, :, c0:c0 + cn].rearrange("p a b -> p b a")
                    yi = Y2[:, 1, :, c0:c0 + cn].rearrange("p a b -> p b a")
                    pzr = psA.tile([128, 512], F32, tag="psA")
                    pzi = psA.tile([128, 512], F32, tag="psA")
                    # same stationary (Chi) back-to-back across the two psums
                    nc.tensor.matmul(pzr[:, :n], fhi3[:, 0:128], yr,
                                     start=True, stop=False)
                    nc.tensor.matmul(pzi[:, :n], fhi3[:, 0:128], yi,
                                     start=True, stop=False)
                    nc.tensor.matmul(pzr[:, :n], fhi3[:, 128:256], yi,
                                     start=False, stop=True)
                    nc.tensor.matmul(pzi[:, :n], fhi3[:, 256:384], yr,
                                     start=False, stop=True)
                    drain(Z[:, 0, c0:c0 + cn, 0:65], pzr[:, :n])
                    drain(Z[:, 1, c0:c0 + cn, 0:65], pzi[:, :n])

                # ---- P2 + IW in c-quarters (batched, double-buffered) ----
                Zf = Z.rearrange("p r c a -> p r (c a)")
                Zpf = Zp.rearrange("p s r a b -> p s r (a b)")
                for q in range(4):
                    s = q % 2
                    nc.sync.dma_start(out=Zp[:, s, 0],
                                      in_=Zf[:, 0, q * 3072:(q + 1) * 3072],
                                      transpose=True)
                    nc.sync.dma_start(out=Zp[:, s, 1],
                                      in_=Zf[:, 1, q * 3072:(q + 1) * 3072],
                                      transpose=True)
                    for k0 in range(0, 6, 2):
                        sla = slice(k0 * 512, (k0 + 1) * 512)
                        slb = slice((k0 + 1) * 512, (k0 + 2) * 512)
                        p5a = psA.tile([128, 512], F32, tag="psA")
                        p5b = psA.tile([128, 512], F32, tag="psA")
                        # pair chunks so each stationary loads once per pair
                        nc.tensor.matmul(p5a[:], fwi2[:, 0:128],
                                         Zpf[0:65, s, 0, sla],
                                         start=True, stop=False)
                        nc.tensor.matmul(p5b[:], fwi2[:, 0:128],
                                         Zpf[0:65, s, 0, slb],
                                         start=True, stop=False)
                        nc.tensor.matmul(p5a[:], fwi2[:, 128:256],
                                         Zpf[0:65, s, 1, sla],
                                         start=False, stop=True)
                        nc.tensor.matmul(p5b[:], fwi2[:, 128:256],
                                         Zpf[0:65, s, 1, slb],
                                         start=False, stop=True)
                        for p5, kk in ((p5a, k0), (p5b, k0 + 1)):
                            ot = stg2.tile([128, 512], BF16, tag="ot")
                            drain(ot, p5[:])
                            nc.gpsimd.dma_start(out=out_ext[b, q * 6 + kk],
                                                in_=ot)

    nc.compile()
    return nc


def kernel(x, w1, b1, w2, b2):
    x = np.ascontiguousarray(x, dtype=np.float32)
    key = "nc"
    if key not in _cache:
        _cache[key] = _build_graph()
    nc = _cache[key]

    in_maps = make_in_maps(x, w1, b1, w2, b2)
    res = run_bass_kernel_spmd(nc, in_maps, core_ids=list(range(NCORES)))
    # device layout [B, 24, w, 512] -> [B, w, c, h] -> [B, h, w, c]
    parts = []
    for i in range(NCORES):
        r = np.asarray(res.results[i]["out"], dtype=np.float32)
        r = r.reshape(B, 24, W, 4, H).transpose(0, 4, 2, 1, 3)
        parts.append(r.reshape(B, H, W, BLK))
    corr = np.concatenate(parts, axis=3)
    return (corr + x).astype(np.float32)


def make_in_maps(x, w1, b1, w2, b2):
    fh, fw, fwi2, fhi3, ones = _build_consts()
    in_maps = []
    for i in range(NCORES):
        w1p, w2p = _pack_mlp(w1, b1, w2, b2, i)
        in_maps.append({
            "x": np.ascontiguousarray(x[:, :, :, i * BLK:(i + 1) * BLK]),
            "fh": fh, "fw": fw, "fwi2": fwi2, "fhi3": fhi3,
            "w1p": w1p, "w2p": w2p, "ones": ones,
            "mlam": np.full((96, 1), -LAM, dtype=np.float32),
        })
    return in_maps



# revision 3
# speedup vs baseline: 1.2068x; 1.2068x over previous
"""AFNO2D layer on 8 TRN2 NeuronCores.

Sharding: channel-block parallel. Core i owns channels [96*i, 96*(i+1)) —
exactly block i of the block-diagonal MLP. No collectives.

v2: Hermitian-symmetric forward DFT + contiguous-stationary S2 + IH with
contiguous moving operand.

Per core, per batch sample (tokens t = hk*65 + wc, NT = 8320):
  S1  H-DFT, kh=0..64 only (real input => Hermitian in kh).
      lhsT=x_c [h,w], rhs=fh2=[Ch|Sh] [128,130] -> psum [w, 130] per ch.
      Drain transposes into ZtT [w, khri(130), c] (strided DVE/ACT write)
      so S2's stationary loads are contiguous.
  S2  W-rDFT per kh-pair (k, 128-k): rows share the products Ztr@{Cw,Sw},
      Zti@{Sw,Cw}. 2 contiguous LDW + 2 MM N=260 per pair ->
      Xri [97, hk, ri, wc] rows k and 128-k.
  L1  MLP layer 1 (bias via ones-row), relu drain; chunks of 7 hk,
      emitted as soon as their S2 pairs are done.
  L2  MLP layer 2, softshrink drain -> O2 [c, ri, wc, hk].
  P1  DMA transpose -> Y2 [hk, ri, wc, c] (shares buffer with ZtT).
  IH  H-iDFT F-stationary, moving = wc-chunks of Y2 (contiguous),
      strided drain -> Z [h, ri, c, wcpad(128)].
  P2  DMA transpose c-quarters -> Zp [wcpad, ri, c24, h]
  IW  W-irDFT F-stationary: lhsT=fwi2=[Cwi|-Swi] -> [w, 512]-chunks -> HBM
Residual add + final transpose run on the host in fp32.
"""
import sys
import types
import numpy as np
import ml_dtypes

# run_bass_kernel_spmd(trace=True) needs this hook module; missing in image.
if "antenv.axon_hooks" not in sys.modules:
    _hooks_mod = types.ModuleType("antenv.axon_hooks")
    _hooks_mod._hook = None
    _hooks_mod.set_axon_ntff_profile_hook = lambda h: setattr(_hooks_mod, "_hook", h)
    _hooks_mod.get_axon_ntff_profile_hook = lambda: _hooks_mod._hook
    sys.modules["antenv.axon_hooks"] = _hooks_mod
    try:
        sys.path.insert(0, "/root/.axon_site")
        from trn_agent_boot.trn_boot import _ntff_profile_via_ctypes
        _hooks_mod._hook = _ntff_profile_via_ctypes("/opt/axon/libaxon_pjrt.so")
    except Exception:
        pass

import concourse.bacc as bacc
import concourse.tile as tile
from concourse import mybir
from concourse.bass_utils import run_bass_kernel_spmd

F32 = mybir.dt.float32
BF16 = mybir.dt.bfloat16

B, H, W, C = 4, 128, 128, 768
Wc = W // 2 + 1            # 65
NCORES, BLK = 8, 96        # channels per core
NT = H * Wc                # 8320 tokens per sample
LAM = 0.01
CHK = 455                  # 7 hk per MLP chunk

_cache = {}


def _build_consts():
    bf = ml_dtypes.bfloat16
    h = np.arange(H)
    k65 = np.arange(Wc)
    wc = np.arange(Wc)
    w = np.arange(W)
    hk = np.arange(H)
    ang_h = 2 * np.pi * np.outer(h, k65) / H
    Ch, Sh = np.cos(ang_h) / np.sqrt(H), -np.sin(ang_h) / np.sqrt(H)
    ang_w = 2 * np.pi * np.outer(w, wc) / W
    Cw, Sw = np.cos(ang_w) / np.sqrt(W), -np.sin(ang_w) / np.sqrt(W)
    alpha = np.ones(Wc)
    alpha[1:64] = 2.0
    ang_wi = 2 * np.pi * np.outer(wc, w) / W
    Cwi = alpha[:, None] * np.cos(ang_wi) / np.sqrt(W)
    Swi = alpha[:, None] * np.sin(ang_wi) / np.sqrt(W)
    ang_hi = 2 * np.pi * np.outer(hk, h) / H
    Chi, Shi = np.cos(ang_hi) / np.sqrt(H), np.sin(ang_hi) / np.sqrt(H)

    fh2 = np.concatenate([Ch, Sh], axis=1).astype(bf)                  # [128,130]
    # S2 pair trick: psum = Ztr@fw4 + Zti@fw4i
    #   cols 0:130   -> row k      (r|i)
    #   cols 130:260 -> row 128-k  (r|i)
    fw4 = np.concatenate([Cw, Sw, Cw, Sw], axis=1).astype(bf)          # [128,260]
    fw4i = np.concatenate([-Sw, Cw, Sw, -Cw], axis=1).astype(bf)       # [128,260]
    # iW (final, real out): out = Cwi^T Zpr + (-Swi)^T Zpi
    fwi2 = np.concatenate([Cwi, -Swi], axis=1).astype(bf)              # [65,256]
    # iH (complex): Zr = Chi^T Yr - Shi^T Yi ; Zi = Shi^T Yr + Chi^T Yi
    fhi3 = np.concatenate([Chi, -Shi, Shi], axis=1).astype(bf)         # [128,384]
    ones = np.ones((1, 2 * NT), dtype=np.float32).astype(bf)           # [1,16640]
    return fh2, fw4, fw4i, fwi2, fhi3, ones


def _pack_mlp(w1, b1, w2, b2, blk):
    """[97, 384] packs: cols [Wr;br | -Wi;0 | Wi;bi | Wr;0]."""
    def pack(wr, wi, br, bi):
        p = np.zeros((97, 384), dtype=np.float32)
        p[:96, 0:96] = wr
        p[96, 0:96] = br
        p[:96, 96:192] = -wi
        p[:96, 192:288] = wi
        p[96, 192:288] = bi
        p[:96, 288:384] = wr
        return p
    w1p = pack(w1[0, blk], w1[1, blk], b1[0, blk], b1[1, blk])
    w2p = pack(w2[0, blk], w2[1, blk], b2[0, blk], b2[1, blk])
    return w1p, w2p


def _build_graph():
    nc = bacc.Bacc("TRN2", target_bir_lowering=False, debug=False,
                   num_devices=NCORES)

    x_ext = nc.dram_tensor("x", [B, H, W, BLK], F32, kind="ExternalInput").ap()
    fh_ext = nc.dram_tensor("fh2", [128, 130], BF16, kind="ExternalInput").ap()
    fw_ext = nc.dram_tensor("fw4", [128, 260], BF16, kind="ExternalInput").ap()
    fwi_ext2 = nc.dram_tensor("fw4i", [128, 260], BF16, kind="ExternalInput").ap()
    fwi_ext = nc.dram_tensor("fwi2", [65, 256], BF16, kind="ExternalInput").ap()
    fhi_ext = nc.dram_tensor("fhi3", [128, 384], BF16, kind="ExternalInput").ap()
    w1_ext = nc.dram_tensor("w1p", [97, 384], F32, kind="ExternalInput").ap()
    w2_ext = nc.dram_tensor("w2p", [97, 384], F32, kind="ExternalInput").ap()
    on_ext = nc.dram_tensor("ones", [1, 2 * NT], BF16, kind="ExternalInput").ap()
    ml_ext = nc.dram_tensor("mlam", [96, 1], F32, kind="ExternalInput").ap()
    # device out: 24 chunks of [w, 512] over (c h)-flat; host reassembles
    out_ext = nc.dram_tensor("out", [B, 24, W, 512], BF16,
                             kind="ExternalOutput").ap()

    SUB = mybir.AluOpType.subtract
    MIN = mybir.AluOpType.min
    MAX = mybir.AluOpType.max
    RELU = mybir.ActivationFunctionType.Relu

    # L1/L2 chunk j is ready after S2 pair max-unit mu(j)
    ready = {}
    for j in range(19):
        rows = range(7 * j, min(7 * j + 7, H))
        mu = max(r if r <= 64 else H - r for r in rows)
        ready.setdefault(mu, []).append(j)

    with tile.TileContext(nc) as tc:
        with (
            tc.tile_pool(name="consts", bufs=1) as cpool,
            tc.tile_pool(name="stat", bufs=1) as spool,
            tc.tile_pool(name="stg", bufs=2) as stg,      # clip staging
            tc.tile_pool(name="stg2", bufs=3) as stg2,    # out staging
            tc.tile_pool(name="psA", bufs=4, space="PSUM") as psA,  # [128,512]
            tc.tile_pool(name="psB", bufs=4, space="PSUM") as psB,  # [96,512]
        ):
            # ---- constants / weights to SBUF (once) ----
            fh2 = cpool.tile([128, 130], BF16, tag="fh2")
            nc.sync.dma_start(out=fh2, in_=fh_ext)
            fw4 = cpool.tile([128, 260], BF16, tag="fw4")
            nc.sync.dma_start(out=fw4, in_=fw_ext)
            fw4i = cpool.tile([128, 260], BF16, tag="fw4i")
            nc.sync.dma_start(out=fw4i, in_=fwi_ext2)
            fwi2 = cpool.tile([65, 256], BF16, tag="fwi2")
            nc.sync.dma_start(out=fwi2, in_=fwi_ext)
            fhi3 = cpool.tile([128, 384], BF16, tag="fhi3")
            nc.sync.dma_start(out=fhi3, in_=fhi_ext)
            w1p = cpool.tile([97, 384], BF16, tag="w1p")
            nc.gpsimd.dma_start(out=w1p, in_=w1_ext)      # casting DMA
            w2p = cpool.tile([97, 384], BF16, tag="w2p")
            nc.gpsimd.dma_start(out=w2p, in_=w2_ext)
            mlam = cpool.tile([96, 1], F32, tag="mlam")
            nc.sync.dma_start(out=mlam, in_=ml_ext)

            # W slices: lhsT [K, 96]
            W1ra = w1p[:, 0:96]          # [97, 96] row96 = b1r
            W1mi = w1p[0:96, 96:192]     # -Wi
            W1ib = w1p[:, 192:288]       # Wi ; b1i
            W1rb = w1p[0:96, 288:384]    # Wr
            W2ra = w2p[:, 0:96]
            W2mi = w2p[0:96, 96:192]
            W2ib = w2p[:, 192:288]
            W2rb = w2p[0:96, 288:384]

            # ---- static tiles (time-shared across stages/samples) ----
            X32f = spool.tile([128, W * BLK], BF16, tag="x32")
            X32 = X32f.rearrange("p (w c) -> p w c", c=BLK)
            # ZtT (S1 out, [w, khri, c]) shares the buffer with Y2
            # (P1 out, [hk, ri, wc, c]): S2 fully consumes ZtT before P1.
            ZTY = spool.tile([128, 130 * BLK], BF16, tag="zty")
            ZtT = ZTY.rearrange("p (k c) -> p k c", c=BLK)        # [128,130,96]
            Y2 = ZTY.rearrange("p (r a c) -> p r a c", r=2, a=Wc)  # [128,2,65,96]
            Xri = spool.tile([97, H, 2, Wc], BF16, tag="xri")      # [97,128,2,65]
            nc.sync.dma_start(out=Xri[96:97, :, :, :],
                              in_=on_ext[:, 0:H * 130])
            O1 = spool.tile([97, 2, 3, CHK], BF16, tag="o1")      # 3-chunk ring
            nc.sync.dma_start(out=O1[96:97, :, :, :],
                              in_=on_ext[:, 0:2 * 3 * CHK])
            # wc-major so P1's DMA-transpose input [96, hk] is contiguous
            O2 = spool.tile([96, 2, Wc, H], BF16, tag="o2")
            # iH out [h, ri, c, wcpad]: wcpad=128 for P2 transpose; pad cols
            # 65:128 are never read downstream (IW reads Zp partitions 0:65).
            Z = spool.tile([128, 2 * BLK * 128], BF16, tag="z")
            Z3 = Z.rearrange("p (r c a) -> p r c a", r=2, c=BLK)   # [128,2,96,128]
            Zp = spool.tile([128, 2, 2, 24, 128], BF16, tag="zp")  # 2 c-quarters
            Zpf = Zp.rearrange("p s r a b -> p s r (a b)")

            for b in range(B):
                # ---- load sample (bufs=1: overwrites after S1(b-1) read) ----
                nc.gpsimd.dma_start(out=X32f, in_=x_ext[b])

                # alternate PSUM-drain engines (Pool cannot read PSUM)
                rr = [0]

                def drain(out, in_):
                    rr[0] += 1
                    if rr[0] % 2:
                        nc.vector.tensor_copy(out, in_)
                    else:
                        nc.scalar.copy(out, in_)

                # ---- S1: 3 channels per psum -> ZtT [w, khri, c] ----
                for c0 in range(0, BLK, 3):
                    p1 = psA.tile([128, 390], F32, tag="psA")
                    for j in range(3):
                        nc.tensor.matmul(p1[:, j * 130:(j + 1) * 130],
                                         X32[:, :, c0 + j], fh2[:],
                                         start=True, stop=True)
                    drain(ZtT[:, :, c0:c0 + 3],
                          p1.rearrange("p (c k) -> p k c", c=3))

                # ---- S2 (pair k,128-k per psum) + L1/L2 interleaved ----
                kchunk = 0
                for k in range(65):
                    ps = psB.tile([96, 260], F32, tag="psB")
                    nc.tensor.matmul(ps[:], ZtT[:, k, :], fw4[:],
                                     start=True, stop=False)
                    nc.tensor.matmul(ps[:], ZtT[:, 65 + k, :], fw4i[:],
                                     start=False, stop=True)
                    drain(Xri[0:96, k, :, :], ps[:, 0:130])
                    if 1 <= k <= 63:
                        drain(Xri[0:96, H - k, :, :], ps[:, 130:260])

                    for j in ready.get(k, []):
                        h0 = 7 * j
                        hn = min(7, H - h0)
                        n = hn * Wc
                        xr = Xri[:, h0:h0 + hn, 0, :]
                        xi = Xri[:, h0:h0 + hn, 1, :]
                        pr = psB.tile([96, CHK], F32, tag="psB")
                        pi = psB.tile([96, CHK], F32, tag="psB")
                        nc.tensor.matmul(pr[:, :n], W1ra, xr, start=True,
                                         stop=False)
                        nc.tensor.matmul(pr[:, :n], W1mi, xi[0:96],
                                         start=False, stop=True)
                        nc.tensor.matmul(pi[:, :n], W1ib, xr, start=True,
                                         stop=False)
                        nc.tensor.matmul(pi[:, :n], W1rb, xi[0:96],
                                         start=False, stop=True)
                        kr = kchunk % 3
                        kchunk += 1
                        nc.vector.tensor_scalar(O1[0:96, 0, kr, :n], pr[:, :n],
                                                0.0, None, MAX)
                        nc.scalar.activation(O1[0:96, 1, kr, :n], pi[:, :n],
                                             RELU)

                        # L2 on the chunk just produced
                        o1r = O1[:, 0, kr, :n]
                        o1i = O1[:, 1, kr, :n]
                        qr = psB.tile([96, CHK], F32, tag="psB")
                        qi = psB.tile([96, CHK], F32, tag="psB")
                        nc.tensor.matmul(qr[:, :n], W2ra, o1r, start=True,
                                         stop=False)
                        nc.tensor.matmul(qr[:, :n], W2mi, o1i[0:96],
                                         start=False, stop=True)
                        nc.tensor.matmul(qi[:, :n], W2ib, o1r, start=True,
                                         stop=False)
                        nc.tensor.matmul(qi[:, :n], W2rb, o1i[0:96],
                                         start=False, stop=True)
                        # O2 dst written contiguously (wc-major); psum operands
                        # read with (wc, hk)-permuted APs instead
                        o2r = O2[:, 0, :, h0:h0 + hn]
                        o2i = O2[:, 1, :, h0:h0 + hn]
                        qrv = qr[:, :n].rearrange("p (a b) -> p b a", b=Wc)
                        qiv = qi[:, :n].rearrange("p (a b) -> p b a", b=Wc)
                        # real: softshrink = y - clip(y) on DVE
                        t1 = stg.tile([96, CHK], F32, tag="clip")
                        t1v = t1[:, :n].rearrange("p (a b) -> p b a", b=Wc)
                        nc.vector.tensor_scalar(t1[:, :n], qr[:, :n], LAM, -LAM,
                                                MIN, MAX)
                        nc.vector.tensor_tensor(o2r, qrv, t1v, SUB)
                        # imag: relu(y-lam) - relu(-y-lam) on Act, sub on Pool
                        sa = stg.tile([96, CHK], BF16, tag="sa")
                        sb = stg.tile([96, CHK], BF16, tag="sb")
                        sav = sa[:, :n].rearrange("p (a b) -> p b a", b=Wc)
                        sbv = sb[:, :n].rearrange("p (a b) -> p b a", b=Wc)
                        nc.scalar.activation(sa[:, :n], qi[:, :n], RELU,
                                             bias=mlam)
                        nc.scalar.activation(sb[:, :n], qi[:, :n], RELU,
                                             bias=mlam, scale=-1.0)
                        nc.gpsimd.tensor_tensor(o2i, sav, sbv, SUB)

                # ---- P1: one batched DMA transpose per r/i ----
                # in [96, (wc.128hk)] -> out [128hk, wc, 96c]
                O2f = O2.rearrange("p r a b -> p r (a b)")
                nc.sync.dma_start(out=Y2[:, 0], in_=O2f[:, 0], transpose=True)
                nc.sync.dma_start(out=Y2[:, 1], in_=O2f[:, 1], transpose=True)

                # ---- IH: wc-chunks of 5 (contiguous moving operand) ----
                for w0 in range(0, Wc, 5):
                    wn = 5
                    n = wn * BLK
                    yr = Y2[:, 0, w0:w0 + wn, :]
                    yi = Y2[:, 1, w0:w0 + wn, :]
                    pzr = psA.tile([128, 480], F32, tag="psA")
                    pzi = psA.tile([128, 480], F32, tag="psA")
                    # same stationary (Chi) back-to-back across the two psums
                    nc.tensor.matmul(pzr[:, :n], fhi3[:, 0:128], yr,
                                     start=True, stop=False)
                    nc.tensor.matmul(pzi[:, :n], fhi3[:, 0:128], yi,
                                     start=True, stop=False)
                    nc.tensor.matmul(pzr[:, :n], fhi3[:, 128:256], yi,
                                     start=False, stop=True)
                    nc.tensor.matmul(pzi[:, :n], fhi3[:, 256:384], yr,
                                     start=False, stop=True)
                    drain(Z3[:, 0, :, w0:w0 + wn],
                          pzr[:, :n].rearrange("p (a b) -> p b a", a=wn))
                    drain(Z3[:, 1, :, w0:w0 + wn],
                          pzi[:, :n].rearrange("p (a b) -> p b a", a=wn))

                # ---- P2 + IW in c-quarters (batched, double-buffered) ----
                Zf = Z3.rearrange("p r c a -> p r (c a)")
                for q in range(4):
                    s = q % 2
                    nc.sync.dma_start(out=Zp[:, s, 0],
                                      in_=Zf[:, 0, q * 3072:(q + 1) * 3072],
                                      transpose=True)
                    nc.sync.dma_start(out=Zp[:, s, 1],
                                      in_=Zf[:, 1, q * 3072:(q + 1) * 3072],
                                      transpose=True)
                    for k0 in range(0, 6, 2):
                        sla = slice(k0 * 512, (k0 + 1) * 512)
                        slb = slice((k0 + 1) * 512, (k0 + 2) * 512)
                        p5a = psA.tile([128, 512], F32, tag="psA")
                        p5b = psA.tile([128, 512], F32, tag="psA")
                        # pair chunks so each stationary loads once per pair
                        nc.tensor.matmul(p5a[:], fwi2[:, 0:128],
                                         Zpf[0:65, s, 0, sla],
                                         start=True, stop=False)
                        nc.tensor.matmul(p5b[:], fwi2[:, 0:128],
                                         Zpf[0:65, s, 0, slb],
                                         start=True, stop=False)
                        nc.tensor.matmul(p5a[:], fwi2[:, 128:256],
                                         Zpf[0:65, s, 1, sla],
                                         start=False, stop=True)
                        nc.tensor.matmul(p5b[:], fwi2[:, 128:256],
                                         Zpf[0:65, s, 1, slb],
                                         start=False, stop=True)
                        for p5, kk in ((p5a, k0), (p5b, k0 + 1)):
                            ot = stg2.tile([128, 512], BF16, tag="ot")
                            drain(ot, p5[:])
                            nc.gpsimd.dma_start(out=out_ext[b, q * 6 + kk],
                                                in_=ot)

    nc.compile()
    return nc


def kernel(x, w1, b1, w2, b2):
    x = np.ascontiguousarray(x, dtype=np.float32)
    key = "nc"
    if key not in _cache:
        _cache[key] = _build_graph()
    nc = _cache[key]

    in_maps = make_in_maps(x, w1, b1, w2, b2)
    res = run_bass_kernel_spmd(nc, in_maps, core_ids=list(range(NCORES)))
    # device layout [B, 24, w, 512] -> [B, w, c, h] -> [B, h, w, c]
    parts = []
    for i in range(NCORES):
        r = np.asarray(res.results[i]["out"], dtype=np.float32)
        r = r.reshape(B, 24, W, 4, H).transpose(0, 4, 2, 1, 3)
        parts.append(r.reshape(B, H, W, BLK))
    corr = np.concatenate(parts, axis=3)
    return (corr + x).astype(np.float32)


def make_in_maps(x, w1, b1, w2, b2):
    fh2, fw4, fw4i, fwi2, fhi3, ones = _build_consts()
    in_maps = []
    for i in range(NCORES):
        w1p, w2p = _pack_mlp(w1, b1, w2, b2, i)
        in_maps.append({
            "x": np.ascontiguousarray(x[:, :, :, i * BLK:(i + 1) * BLK]),
            "fh2": fh2, "fw4": fw4, "fw4i": fw4i, "fwi2": fwi2, "fhi3": fhi3,
            "w1p": w1p, "w2p": w2p, "ones": ones,
            "mlam": np.full((96, 1), -LAM, dtype=np.float32),
        })
    return in_maps


# revision 5
# speedup vs baseline: 1.2141x; 1.0061x over previous
"""AFNO2D layer on 8 TRN2 NeuronCores.

Sharding: channel-block parallel. Core i owns channels [96*i, 96*(i+1)) —
exactly block i of the block-diagonal MLP. No collectives.

v3: Hermitian forward DFT, contiguous-stationary S2 with paired single
drains, IH with contiguous moving operand, and cross-sample overlap
(sample b+1's S1/S2 fills the PE idle while sample b runs P1/IH/P2/IW).

Per core, per batch sample (tokens t = hk*65 + wc, NT = 8320):
  S1  H-DFT, kh=0..64 only (real input => Hermitian in kh).
      lhsT=x_c [h,w], rhs=fh2=[Ch|Sh] [128,130] -> psum [w, 130] per ch.
      Drain transposes into ZtT [w, khri(130), c] (strided DVE/ACT write)
      so S2's stationary loads are contiguous. Own PSUM pool (psC) so it
      can run while the previous sample's inverse phase occupies psA.
  S2  W-rDFT per kh-pair (k, 128-k): rows share the products Ztr@{Cw,Sw},
      Zti@{Sw,Cw}: 2 contiguous LDW + 2 MM N=260 per pair. One drain per
      pair via a step-sliced Xri view covering rows k and 128-k.
  L1  MLP layer 1 (bias via ones-row), relu drain; chunks of 7 hk,
      emitted as soon as their S2 pairs are done.
  L2  MLP layer 2, softshrink drain -> O2 [c, ri, wc, hk].
  P1  DMA transpose -> Y2 [hk, ri, wc, c]. Y2 lives inside the Xri
      buffer (Xri is fully consumed before P1 writes); the ones-row
      (partition 96) is re-DMAed after IH reads.
  IH  H-iDFT F-stationary, moving = wc-chunks of Y2 (contiguous),
      strided drain -> Z [h, ri, c, wcpad(128)].
  P2  DMA transpose c-eighths -> Zp [wcpad, ri, c12, h] (double-buffered)
  IW  W-irDFT F-stationary: lhsT=fwi2=[Cwi|-Swi] -> [w, 512]-chunks -> HBM
Residual add + final transpose run on the host in fp32.
"""
import sys
import types
import numpy as np
import ml_dtypes

# run_bass_kernel_spmd(trace=True) needs this hook module; missing in image.
if "antenv.axon_hooks" not in sys.modules:
    _hooks_mod = types.ModuleType("antenv.axon_hooks")
    _hooks_mod._hook = None
    _hooks_mod.set_axon_ntff_profile_hook = lambda h: setattr(_hooks_mod, "_hook", h)
    _hooks_mod.get_axon_ntff_profile_hook = lambda: _hooks_mod._hook
    sys.modules["antenv.axon_hooks"] = _hooks_mod
    try:
        sys.path.insert(0, "/root/.axon_site")
        from trn_agent_boot.trn_boot import _ntff_profile_via_ctypes
        _hooks_mod._hook = _ntff_profile_via_ctypes("/opt/axon/libaxon_pjrt.so")
    except Exception:
        pass

import concourse.bacc as bacc
import concourse.tile as tile
from concourse import mybir
from concourse.bass_utils import run_bass_kernel_spmd

F32 = mybir.dt.float32
BF16 = mybir.dt.bfloat16

B, H, W, C = 4, 128, 128, 768
Wc = W // 2 + 1            # 65
NCORES, BLK = 8, 96        # channels per core
NT = H * Wc                # 8320 tokens per sample
LAM = 0.01
CHK = 455                  # 7 hk per MLP chunk

_cache = {}


def _build_consts():
    bf = ml_dtypes.bfloat16
    h = np.arange(H)
    k65 = np.arange(Wc)
    wc = np.arange(Wc)
    w = np.arange(W)
    hk = np.arange(H)
    ang_h = 2 * np.pi * np.outer(h, k65) / H
    Ch, Sh = np.cos(ang_h) / np.sqrt(H), -np.sin(ang_h) / np.sqrt(H)
    ang_w = 2 * np.pi * np.outer(w, wc) / W
    Cw, Sw = np.cos(ang_w) / np.sqrt(W), -np.sin(ang_w) / np.sqrt(W)
    alpha = np.ones(Wc)
    alpha[1:64] = 2.0
    ang_wi = 2 * np.pi * np.outer(wc, w) / W
    Cwi = alpha[:, None] * np.cos(ang_wi) / np.sqrt(W)
    Swi = alpha[:, None] * np.sin(ang_wi) / np.sqrt(W)
    ang_hi = 2 * np.pi * np.outer(hk, h) / H
    Chi, Shi = np.cos(ang_hi) / np.sqrt(H), np.sin(ang_hi) / np.sqrt(H)

    fh2 = np.concatenate([Ch, Sh], axis=1).astype(bf)                  # [128,130]
    # S2 pair trick: psum = Ztr@fw4 + Zti@fw4i
    #   cols 0:130   -> row k      (r|i)
    #   cols 130:260 -> row 128-k  (r|i)
    fw4 = np.concatenate([Cw, Sw, Cw, Sw], axis=1).astype(bf)          # [128,260]
    fw4i = np.concatenate([-Sw, Cw, Sw, -Cw], axis=1).astype(bf)       # [128,260]
    # iW (final, real out): out = Cwi^T Zpr + (-Swi)^T Zpi
    fwi2 = np.concatenate([Cwi, -Swi], axis=1).astype(bf)              # [65,256]
    # iH (complex): Zr = Chi^T Yr - Shi^T Yi ; Zi = Shi^T Yr + Chi^T Yi
    fhi3 = np.concatenate([Chi, -Shi, Shi], axis=1).astype(bf)         # [128,384]
    ones = np.ones((1, 2 * NT), dtype=np.float32).astype(bf)           # [1,16640]
    return fh2, fw4, fw4i, fwi2, fhi3, ones


def _pack_mlp(w1, b1, w2, b2, blk):
    """[97, 384] packs: cols [Wr;br | -Wi;0 | Wi;bi | Wr;0]."""
    def pack(wr, wi, br, bi):
        p = np.zeros((97, 384), dtype=np.float32)
        p[:96, 0:96] = wr
        p[96, 0:96] = br
        p[:96, 96:192] = -wi
        p[:96, 192:288] = wi
        p[96, 192:288] = bi
        p[:96, 288:384] = wr
        return p
    w1p = pack(w1[0, blk], w1[1, blk], b1[0, blk], b1[1, blk])
    w2p = pack(w2[0, blk], w2[1, blk], b2[0, blk], b2[1, blk])
    return w1p, w2p


def _build_graph():
    nc = bacc.Bacc("TRN2", target_bir_lowering=False, debug=False,
                   num_devices=NCORES)

    x_ext = nc.dram_tensor("x", [B, H, W, BLK], F32, kind="ExternalInput").ap()
    fh_ext = nc.dram_tensor("fh2", [128, 130], BF16, kind="ExternalInput").ap()
    fw_ext = nc.dram_tensor("fw4", [128, 260], BF16, kind="ExternalInput").ap()
    fwi_ext2 = nc.dram_tensor("fw4i", [128, 260], BF16, kind="ExternalInput").ap()
    fwi_ext = nc.dram_tensor("fwi2", [65, 256], BF16, kind="ExternalInput").ap()
    fhi_ext = nc.dram_tensor("fhi3", [128, 384], BF16, kind="ExternalInput").ap()
    w1_ext = nc.dram_tensor("w1p", [97, 384], F32, kind="ExternalInput").ap()
    w2_ext = nc.dram_tensor("w2p", [97, 384], F32, kind="ExternalInput").ap()
    on_ext = nc.dram_tensor("ones", [1, 2 * NT], BF16, kind="ExternalInput").ap()
    ml_ext = nc.dram_tensor("mlam", [96, 1], F32, kind="ExternalInput").ap()
    # device out: 24 chunks of [w, 512] over (c h)-flat; host reassembles
    out_ext = nc.dram_tensor("out", [B, 24, W, 512], BF16,
                             kind="ExternalOutput").ap()

    SUB = mybir.AluOpType.subtract
    MIN = mybir.AluOpType.min
    MAX = mybir.AluOpType.max
    RELU = mybir.ActivationFunctionType.Relu

    # L1/L2 chunk j is ready after S2 pair max-unit mu(j)
    ready = {}
    for j in range(19):
        rows = range(7 * j, min(7 * j + 7, H))
        mu = max(r if r <= 64 else H - r for r in rows)
        ready.setdefault(mu, []).append(j)

    with tile.TileContext(nc) as tc:
        with (
            tc.tile_pool(name="consts", bufs=1) as cpool,
            tc.tile_pool(name="stat", bufs=1) as spool,
            tc.tile_pool(name="stg", bufs=2) as stg,      # clip staging
            tc.tile_pool(name="stg2", bufs=3) as stg2,    # out staging
            tc.tile_pool(name="psA", bufs=3, space="PSUM") as psA,  # IH/IW
            tc.tile_pool(name="psB", bufs=3, space="PSUM") as psB,  # S2/L1/L2
            tc.tile_pool(name="psC", bufs=2, space="PSUM") as psC,  # S1
        ):
            # ---- constants / weights to SBUF (once) ----
            fh2 = cpool.tile([128, 130], BF16, tag="fh2")
            nc.sync.dma_start(out=fh2, in_=fh_ext)
            fw4 = cpool.tile([128, 260], BF16, tag="fw4")
            nc.sync.dma_start(out=fw4, in_=fw_ext)
            fw4i = cpool.tile([128, 260], BF16, tag="fw4i")
            nc.sync.dma_start(out=fw4i, in_=fwi_ext2)
            fwi2 = cpool.tile([65, 256], BF16, tag="fwi2")
            nc.sync.dma_start(out=fwi2, in_=fwi_ext)
            fhi3 = cpool.tile([128, 384], BF16, tag="fhi3")
            nc.sync.dma_start(out=fhi3, in_=fhi_ext)
            w1p = cpool.tile([97, 384], BF16, tag="w1p")
            nc.gpsimd.dma_start(out=w1p, in_=w1_ext)      # casting DMA
            w2p = cpool.tile([97, 384], BF16, tag="w2p")
            nc.gpsimd.dma_start(out=w2p, in_=w2_ext)
            mlam = cpool.tile([96, 1], F32, tag="mlam")
            nc.sync.dma_start(out=mlam, in_=ml_ext)

            # W slices: lhsT [K, 96]
            W1ra = w1p[:, 0:96]          # [97, 96] row96 = b1r
            W1mi = w1p[0:96, 96:192]     # -Wi
            W1ib = w1p[:, 192:288]       # Wi ; b1i
            W1rb = w1p[0:96, 288:384]    # Wr
            W2ra = w2p[:, 0:96]
            W2mi = w2p[0:96, 96:192]
            W2ib = w2p[:, 192:288]
            W2rb = w2p[0:96, 288:384]

            # ---- static tiles (time-shared across stages/samples) ----
            X32f = spool.tile([128, W * BLK], BF16, tag="x32")
            X32 = X32f.rearrange("p (w c) -> p w c", c=BLK)
            ZtTf = spool.tile([128, 130 * BLK], BF16, tag="ztt")
            ZtT = ZtTf.rearrange("p (k c) -> p k c", c=BLK)        # [128,130,96]
            # Xri [97(+ones row), hk, ri, wc] shares its buffer with Y2
            # (P1 out, [hk, ri, wc, c]): L1 fully consumes Xri before P1
            # writes; the ones row (partition 96) is re-DMAed after IH.
            SHR = spool.tile([128, H * 130], BF16, tag="shr")
            Xri = SHR.rearrange("p (h r a) -> p h r a", h=H, r=2)  # [128,128,2,65]
            Y2 = SHR[:, 0:2 * Wc * BLK].rearrange(
                "p (r a c) -> p r a c", r=2, a=Wc)                 # [128,2,65,96]
            nc.sync.dma_start(out=Xri[96:97, :, :, :],
                              in_=on_ext[:, 0:H * 130])
            O1 = spool.tile([97, 2, 3, CHK], BF16, tag="o1")      # 3-chunk ring
            nc.sync.dma_start(out=O1[96:97, :, :, :],
                              in_=on_ext[:, 0:2 * 3 * CHK])
            # wc-major so P1's DMA-transpose input [96, hk] is contiguous
            O2 = spool.tile([96, 2, Wc, H], BF16, tag="o2")
            # iH out [h, ri, c, wcpad]: wcpad=128 for P2 transpose; pad cols
            # 65:128 are never read downstream (IW reads Zp partitions 0:65).
            Z = spool.tile([128, 2 * BLK * 128], BF16, tag="z")
            Z3 = Z.rearrange("p (r c a) -> p r c a", r=2, c=BLK)   # [128,2,96,128]
            Zp = spool.tile([128, 2, 2, 12, 128], BF16, tag="zp")  # 2 c-eighths
            Zpf = Zp.rearrange("p s r a b -> p s r (a b)")

            for b in range(B):
                # ---- load sample (bufs=1: overwrites after S1(b-1) read) ----
                nc.gpsimd.dma_start(out=X32f, in_=x_ext[b])

                # alternate PSUM-drain engines (Pool cannot read PSUM)
                rr = [0]

                def drain(out, in_):
                    rr[0] += 1
                    if rr[0] % 2:
                        nc.vector.tensor_copy(out, in_)
                    else:
                        nc.scalar.copy(out, in_)

                # ---- S1: 3 channels per psum -> ZtT [w, khri, c] ----
                for c0 in range(0, BLK, 3):
                    p1 = psC.tile([128, 390], F32, tag="psC")
                    for j in range(3):
                        nc.tensor.matmul(p1[:, j * 130:(j + 1) * 130],
                                         X32[:, :, c0 + j], fh2[:],
                                         start=True, stop=True)
                    drain(ZtT[:, :, c0:c0 + 3],
                          p1.rearrange("p (c k) -> p k c", c=3))

                # ---- S2 (pair k,128-k per psum) + L1/L2 interleaved ----
                kchunk = 0
                for k in range(65):
                    ps = psB.tile([96, 260], F32, tag="psB")
                    nc.tensor.matmul(ps[:], ZtT[:, k, :], fw4[:],
                                     start=True, stop=False)
                    nc.tensor.matmul(ps[:], ZtT[:, 65 + k, :], fw4i[:],
                                     start=False, stop=True)
                    if 1 <= k <= 63:
                        # one drain for rows k and 128-k via step-slice
                        dst = Xri[0:96, k:129 - k:128 - 2 * k, :, :]
                        drain(dst, ps.rearrange("p (g r a) -> p g r a",
                                                g=2, r=2))
                    else:
                        drain(Xri[0:96, k, :, :], ps[:, 0:130])

                    for j in ready.get(k, []):
                        h0 = 7 * j
                        hn = min(7, H - h0)
                        n = hn * Wc
                        xr = Xri[0:97, h0:h0 + hn, 0, :]
                        xi = Xri[0:97, h0:h0 + hn, 1, :]
                        pr = psB.tile([96, CHK], F32, tag="psB")
                        pi = psB.tile([96, CHK], F32, tag="psB")
                        nc.tensor.matmul(pr[:, :n], W1ra, xr, start=True,
                                         stop=False)
                        nc.tensor.matmul(pr[:, :n], W1mi, xi[0:96],
                                         start=False, stop=True)
                        nc.tensor.matmul(pi[:, :n], W1ib, xr, start=True,
                                         stop=False)
                        nc.tensor.matmul(pi[:, :n], W1rb, xi[0:96],
                                         start=False, stop=True)
                        kr = kchunk % 3
                        kchunk += 1
                        nc.vector.tensor_scalar(O1[0:96, 0, kr, :n], pr[:, :n],
                                                0.0, None, MAX)
                        nc.scalar.activation(O1[0:96, 1, kr, :n], pi[:, :n],
                                             RELU)

                        # L2 on the chunk just produced
                        o1r = O1[:, 0, kr, :n]
                        o1i = O1[:, 1, kr, :n]
                        qr = psB.tile([96, CHK], F32, tag="psB")
                        qi = psB.tile([96, CHK], F32, tag="psB")
                        nc.tensor.matmul(qr[:, :n], W2ra, o1r, start=True,
                                         stop=False)
                        nc.tensor.matmul(qr[:, :n], W2mi, o1i[0:96],
                                         start=False, stop=True)
                        nc.tensor.matmul(qi[:, :n], W2ib, o1r, start=True,
                                         stop=False)
                        nc.tensor.matmul(qi[:, :n], W2rb, o1i[0:96],
                                         start=False, stop=True)
                        # O2 dst written contiguously (wc-major); psum operands
                        # read with (wc, hk)-permuted APs instead
                        o2r = O2[:, 0, :, h0:h0 + hn]
                        o2i = O2[:, 1, :, h0:h0 + hn]
                        qrv = qr[:, :n].rearrange("p (a b) -> p b a", b=Wc)
                        qiv = qi[:, :n].rearrange("p (a b) -> p b a", b=Wc)
                        # real: softshrink = y - clip(y) on DVE
                        t1 = stg.tile([96, CHK], F32, tag="clip")
                        t1v = t1[:, :n].rearrange("p (a b) -> p b a", b=Wc)
                        nc.vector.tensor_scalar(t1[:, :n], qr[:, :n], LAM, -LAM,
                                                MIN, MAX)
                        nc.vector.tensor_tensor(o2r, qrv, t1v, SUB)
                        # imag: relu(y-lam) - relu(-y-lam) on Act, sub on Pool
                        sa = stg.tile([96, CHK], BF16, tag="sa")
                        sb = stg.tile([96, CHK], BF16, tag="sb")
                        sav = sa[:, :n].rearrange("p (a b) -> p b a", b=Wc)
                        sbv = sb[:, :n].rearrange("p (a b) -> p b a", b=Wc)
                        nc.scalar.activation(sa[:, :n], qi[:, :n], RELU,
                                             bias=mlam)
                        nc.scalar.activation(sb[:, :n], qi[:, :n], RELU,
                                             bias=mlam, scale=-1.0)
                        nc.gpsimd.tensor_tensor(o2i, sav, sbv, SUB)

                # ---- P1: one batched DMA transpose per r/i ----
                # in [96, (wc.128hk)] -> out [128hk, wc, 96c]
                O2f = O2.rearrange("p r a b -> p r (a b)")
                nc.sync.dma_start(out=Y2[:, 0], in_=O2f[:, 0], transpose=True)
                nc.sync.dma_start(out=Y2[:, 1], in_=O2f[:, 1], transpose=True)

                # ---- IH: wc-chunks of 5 (contiguous moving operand) ----
                for w0 in range(0, Wc, 5):
                    wn = 5
                    n = wn * BLK
                    yr = Y2[:, 0, w0:w0 + wn, :]
                    yi = Y2[:, 1, w0:w0 + wn, :]
                    pzr = psA.tile([128, 480], F32, tag="psA")
                    pzi = psA.tile([128, 480], F32, tag="psA")
                    # same stationary (Chi) back-to-back across the two psums
                    nc.tensor.matmul(pzr[:, :n], fhi3[:, 0:128], yr,
                                     start=True, stop=False)
                    nc.tensor.matmul(pzi[:, :n], fhi3[:, 0:128], yi,
                                     start=True, stop=False)
                    nc.tensor.matmul(pzr[:, :n], fhi3[:, 128:256], yi,
                                     start=False, stop=True)
                    nc.tensor.matmul(pzi[:, :n], fhi3[:, 256:384], yr,
                                     start=False, stop=True)
                    drain(Z3[:, 0, :, w0:w0 + wn],
                          pzr[:, :n].rearrange("p (a b) -> p b a", a=wn))
                    drain(Z3[:, 1, :, w0:w0 + wn],
                          pzi[:, :n].rearrange("p (a b) -> p b a", a=wn))

                # restore the ones row (P1 overwrote partition 96 of SHR)
                nc.sync.dma_start(out=Xri[96:97, :, :, :],
                                  in_=on_ext[:, 0:H * 130])

                # ---- P2 + IW in c-eighths (batched, double-buffered) ----
                Zf = Z3.rearrange("p r c a -> p r (c a)")
                for e in range(8):
                    s = e % 2
                    nc.sync.dma_start(out=Zp[:, s, 0],
                                      in_=Zf[:, 0, e * 1536:(e + 1) * 1536],
                                      transpose=True)
                    nc.sync.dma_start(out=Zp[:, s, 1],
                                      in_=Zf[:, 1, e * 1536:(e + 1) * 1536],
                                      transpose=True)
                    # 3 chunks of 512; first two share stationary loads
                    p5a = psA.tile([128, 512], F32, tag="psA")
                    p5b = psA.tile([128, 512], F32, tag="psA")
                    sla, slb = slice(0, 512), slice(512, 1024)
                    nc.tensor.matmul(p5a[:], fwi2[:, 0:128],
                                     Zpf[0:65, s, 0, sla],
                                     start=True, stop=False)
                    nc.tensor.matmul(p5b[:], fwi2[:, 0:128],
                                     Zpf[0:65, s, 0, slb],
                                     start=True, stop=False)
                    nc.tensor.matmul(p5a[:], fwi2[:, 128:256],
                                     Zpf[0:65, s, 1, sla],
                                     start=False, stop=True)
                    nc.tensor.matmul(p5b[:], fwi2[:, 128:256],
                                     Zpf[0:65, s, 1, slb],
                                     start=False, stop=True)
                    p5c = psA.tile([128, 512], F32, tag="psA")
                    slc = slice(1024, 1536)
                    nc.tensor.matmul(p5c[:], fwi2[:, 0:128],
                                     Zpf[0:65, s, 0, slc],
                                     start=True, stop=False)
                    nc.tensor.matmul(p5c[:], fwi2[:, 128:256],
                                     Zpf[0:65, s, 1, slc],
                                     start=False, stop=True)
                    for p5, kk in ((p5a, 0), (p5b, 1), (p5c, 2)):
                        ot = stg2.tile([128, 512], BF16, tag="ot")
                        drain(ot, p5[:])
                        nc.gpsimd.dma_start(out=out_ext[b, e * 3 + kk],
                                            in_=ot)

    nc.compile()
    return nc


def kernel(x, w1, b1, w2, b2):
    x = np.ascontiguousarray(x, dtype=np.float32)
    key = "nc"
    if key not in _cache:
        _cache[key] = _build_graph()
    nc = _cache[key]

    in_maps = make_in_maps(x, w1, b1, w2, b2)
    res = run_bass_kernel_spmd(nc, in_maps, core_ids=list(range(NCORES)))
    # device layout [B, 24, w, 512] -> [B, w, c, h] -> [B, h, w, c]
    parts = []
    for i in range(NCORES):
        r = np.asarray(res.results[i]["out"], dtype=np.float32)
        r = r.reshape(B, 24, W, 4, H).transpose(0, 4, 2, 1, 3)
        parts.append(r.reshape(B, H, W, BLK))
    corr = np.concatenate(parts, axis=3)
    return (corr + x).astype(np.float32)


def make_in_maps(x, w1, b1, w2, b2):
    fh2, fw4, fw4i, fwi2, fhi3, ones = _build_consts()
    in_maps = []
    for i in range(NCORES):
        w1p, w2p = _pack_mlp(w1, b1, w2, b2, i)
        in_maps.append({
            "x": np.ascontiguousarray(x[:, :, :, i * BLK:(i + 1) * BLK]),
            "fh2": fh2, "fw4": fw4, "fw4i": fw4i, "fwi2": fwi2, "fhi3": fhi3,
            "w1p": w1p, "w2p": w2p, "ones": ones,
            "mlam": np.full((96, 1), -LAM, dtype=np.float32),
        })
    return in_maps


# revision 6
# speedup vs baseline: 1.5092x; 1.2430x over previous
"""AFNO2D layer on 8 TRN2 NeuronCores.

Sharding: channel-block parallel. Core i owns channels [96*i, 96*(i+1)) —
exactly block i of the block-diagonal MLP. No collectives.

v3: Hermitian forward DFT, contiguous-stationary S2 with paired single
drains, IH with contiguous moving operand, and cross-sample overlap
(sample b+1's S1/S2 fills the PE idle while sample b runs P1/IH/P2/IW).

Per core, per batch sample (tokens t = hk*65 + wc, NT = 8320):
  S1  H-DFT, kh=0..64 only (real input => Hermitian in kh).
      lhsT=x_c [h,w], rhs=fh2=[Ch|Sh] [128,130] -> psum [w, 130] per ch.
      Drain transposes into ZtT [w, khri(130), c] (strided DVE/ACT write)
      so S2's stationary loads are contiguous. Own PSUM pool (psC) so it
      can run while the previous sample's inverse phase occupies psA.
  S2  W-rDFT per kh-pair (k, 128-k): rows share the products Ztr@{Cw,Sw},
      Zti@{Sw,Cw}: 2 contiguous LDW + 2 MM N=260 per pair. One drain per
      pair via a step-sliced Xri view covering rows k and 128-k.
  L1  MLP layer 1 (bias via ones-row), relu drain; chunks of 7 hk,
      emitted as soon as their S2 pairs are done.
  L2  MLP layer 2, softshrink drain -> O2 [c, ri, wc, hk].
  P1  DMA transpose -> Y2 [hk, ri, wc, c]. Y2 lives inside the Xri
      buffer (Xri is fully consumed before P1 writes); the ones-row
      (partition 96) is re-DMAed after IH reads.
  IH  H-iDFT F-stationary, moving = wc-chunks of Y2 (contiguous),
      strided drain -> Z [h, ri, c, wcpad(128)].
  P2  DMA transpose c-eighths -> Zp [wcpad, ri, c12, h] (double-buffered)
  IW  W-irDFT F-stationary: lhsT=fwi2=[Cwi|-Swi] -> [w, 512]-chunks -> HBM
Residual add + final transpose run on the host in fp32.
"""
import sys
import types
import numpy as np
import ml_dtypes

# run_bass_kernel_spmd(trace=True) needs this hook module; missing in image.
if "antenv.axon_hooks" not in sys.modules:
    _hooks_mod = types.ModuleType("antenv.axon_hooks")
    _hooks_mod._hook = None
    _hooks_mod.set_axon_ntff_profile_hook = lambda h: setattr(_hooks_mod, "_hook", h)
    _hooks_mod.get_axon_ntff_profile_hook = lambda: _hooks_mod._hook
    sys.modules["antenv.axon_hooks"] = _hooks_mod
    try:
        sys.path.insert(0, "/root/.axon_site")
        from trn_agent_boot.trn_boot import _ntff_profile_via_ctypes
        _hooks_mod._hook = _ntff_profile_via_ctypes("/opt/axon/libaxon_pjrt.so")
    except Exception:
        pass

import concourse.bacc as bacc
import concourse.tile as tile
from concourse import mybir
from concourse.bass_utils import run_bass_kernel_spmd

F32 = mybir.dt.float32
BF16 = mybir.dt.bfloat16

B, H, W, C = 4, 128, 128, 768
Wc = W // 2 + 1            # 65
NCORES, BLK = 8, 96        # channels per core
NT = H * Wc                # 8320 tokens per sample
LAM = 0.01
CHK = 455                  # 7 hk per MLP chunk

_cache = {}


def _build_consts():
    bf = ml_dtypes.bfloat16
    h = np.arange(H)
    k65 = np.arange(Wc)
    wc = np.arange(Wc)
    w = np.arange(W)
    hk = np.arange(H)
    ang_h = 2 * np.pi * np.outer(h, k65) / H
    Ch, Sh = np.cos(ang_h) / np.sqrt(H), -np.sin(ang_h) / np.sqrt(H)
    ang_w = 2 * np.pi * np.outer(w, wc) / W
    Cw, Sw = np.cos(ang_w) / np.sqrt(W), -np.sin(ang_w) / np.sqrt(W)
    alpha = np.ones(Wc)
    alpha[1:64] = 2.0
    ang_wi = 2 * np.pi * np.outer(wc, w) / W
    Cwi = alpha[:, None] * np.cos(ang_wi) / np.sqrt(W)
    Swi = alpha[:, None] * np.sin(ang_wi) / np.sqrt(W)
    ang_hi = 2 * np.pi * np.outer(hk, h) / H
    Chi, Shi = np.cos(ang_hi) / np.sqrt(H), np.sin(ang_hi) / np.sqrt(H)

    fh2 = np.concatenate([Ch, Sh], axis=1).astype(bf)                  # [128,130]
    # S2 pair trick: psum = Ztr@fw4 + Zti@fw4i
    #   cols 0:130   -> row k      (r|i)
    #   cols 130:260 -> row 128-k  (r|i)
    fw4 = np.concatenate([Cw, Sw, Cw, Sw], axis=1).astype(bf)          # [128,260]
    fw4i = np.concatenate([-Sw, Cw, Sw, -Cw], axis=1).astype(bf)       # [128,260]
    # iW (final, real out): out = Cwi^T Zpr + (-Swi)^T Zpi
    fwi2 = np.concatenate([Cwi, -Swi], axis=1).astype(bf)              # [65,256]
    # iH (complex): Zr = Chi^T Yr - Shi^T Yi ; Zi = Shi^T Yr + Chi^T Yi
    fhi3 = np.concatenate([Chi, -Shi, Shi], axis=1).astype(bf)         # [128,384]
    ones = np.ones((1, 2 * NT), dtype=np.float32).astype(bf)           # [1,16640]
    return fh2, fw4, fw4i, fwi2, fhi3, ones


def _pack_mlp(w1, b1, w2, b2, blk):
    """[97, 384] packs: cols [Wr;br | -Wi;0 | Wi;bi | Wr;0]."""
    def pack(wr, wi, br, bi):
        p = np.zeros((97, 384), dtype=np.float32)
        p[:96, 0:96] = wr
        p[96, 0:96] = br
        p[:96, 96:192] = -wi
        p[:96, 192:288] = wi
        p[96, 192:288] = bi
        p[:96, 288:384] = wr
        return p
    w1p = pack(w1[0, blk], w1[1, blk], b1[0, blk], b1[1, blk])
    w2p = pack(w2[0, blk], w2[1, blk], b2[0, blk], b2[1, blk])
    return w1p, w2p


def _build_graph():
    nc = bacc.Bacc("TRN2", target_bir_lowering=False, debug=False,
                   num_devices=NCORES)

    x_ext = nc.dram_tensor("x", [B, H, W, BLK], F32, kind="ExternalInput").ap()
    fh_ext = nc.dram_tensor("fh2", [128, 130], BF16, kind="ExternalInput").ap()
    fw_ext = nc.dram_tensor("fw4", [128, 260], BF16, kind="ExternalInput").ap()
    fwi_ext2 = nc.dram_tensor("fw4i", [128, 260], BF16, kind="ExternalInput").ap()
    fwi_ext = nc.dram_tensor("fwi2", [65, 256], BF16, kind="ExternalInput").ap()
    fhi_ext = nc.dram_tensor("fhi3", [128, 384], BF16, kind="ExternalInput").ap()
    w1_ext = nc.dram_tensor("w1p", [97, 384], F32, kind="ExternalInput").ap()
    w2_ext = nc.dram_tensor("w2p", [97, 384], F32, kind="ExternalInput").ap()
    on_ext = nc.dram_tensor("ones", [1, 2 * NT], BF16, kind="ExternalInput").ap()
    ml_ext = nc.dram_tensor("mlam", [96, 1], F32, kind="ExternalInput").ap()
    # device out: 24 chunks of [w, 512] over (c h)-flat; host reassembles
    out_ext = nc.dram_tensor("out", [B, 24, W, 512], BF16,
                             kind="ExternalOutput").ap()

    SUB = mybir.AluOpType.subtract
    MIN = mybir.AluOpType.min
    MAX = mybir.AluOpType.max
    RELU = mybir.ActivationFunctionType.Relu

    # L1/L2 chunk j is ready after S2 pair max-unit mu(j)
    ready = {}
    for j in range(19):
        rows = range(7 * j, min(7 * j + 7, H))
        mu = max(r if r <= 64 else H - r for r in rows)
        ready.setdefault(mu, []).append(j)

    with tile.TileContext(nc) as tc:
        with (
            tc.tile_pool(name="consts", bufs=1) as cpool,
            tc.tile_pool(name="stat", bufs=1) as spool,
            tc.tile_pool(name="stg", bufs=2) as stg,      # clip staging
            tc.tile_pool(name="stg2", bufs=3) as stg2,    # out staging
            tc.tile_pool(name="psA", bufs=3, space="PSUM") as psA,  # IH/IW
            tc.tile_pool(name="psB", bufs=3, space="PSUM") as psB,  # S2/L1/L2
            tc.tile_pool(name="psC", bufs=2, space="PSUM") as psC,  # S1
        ):
            # ---- constants / weights to SBUF (once) ----
            fh2 = cpool.tile([128, 130], BF16, tag="fh2")
            nc.sync.dma_start(out=fh2, in_=fh_ext)
            fw4 = cpool.tile([128, 260], BF16, tag="fw4")
            nc.sync.dma_start(out=fw4, in_=fw_ext)
            fw4i = cpool.tile([128, 260], BF16, tag="fw4i")
            nc.sync.dma_start(out=fw4i, in_=fwi_ext2)
            fwi2 = cpool.tile([65, 256], BF16, tag="fwi2")
            nc.sync.dma_start(out=fwi2, in_=fwi_ext)
            fhi3 = cpool.tile([128, 384], BF16, tag="fhi3")
            nc.sync.dma_start(out=fhi3, in_=fhi_ext)
            w1p = cpool.tile([97, 384], BF16, tag="w1p")
            nc.gpsimd.dma_start(out=w1p, in_=w1_ext)      # casting DMA
            w2p = cpool.tile([97, 384], BF16, tag="w2p")
            nc.gpsimd.dma_start(out=w2p, in_=w2_ext)
            mlam = cpool.tile([96, 1], F32, tag="mlam")
            nc.sync.dma_start(out=mlam, in_=ml_ext)

            # W slices: lhsT [K, 96]
            W1ra = w1p[:, 0:96]          # [97, 96] row96 = b1r
            W1mi = w1p[0:96, 96:192]     # -Wi
            W1ib = w1p[:, 192:288]       # Wi ; b1i
            W1rb = w1p[0:96, 288:384]    # Wr
            W2ra = w2p[:, 0:96]
            W2mi = w2p[0:96, 96:192]
            W2ib = w2p[:, 192:288]
            W2rb = w2p[0:96, 288:384]

            # ---- static tiles (time-shared across stages/samples) ----
            X32f = spool.tile([128, W * BLK], BF16, tag="x32")
            X32 = X32f.rearrange("p (w c) -> p w c", c=BLK)
            ZtTf = spool.tile([128, 130 * BLK], BF16, tag="ztt")
            ZtT = ZtTf.rearrange("p (k c) -> p k c", c=BLK)        # [128,130,96]
            # Xri [97(+ones row), hk, ri, wc] shares its buffer with Y2
            # (P1 out, [hk, ri, wc, c]): L1 fully consumes Xri before P1
            # writes; the ones row (partition 96) is re-DMAed after IH.
            SHR = spool.tile([128, H * 130], BF16, tag="shr")
            Xri = SHR.rearrange("p (h r a) -> p h r a", h=H, r=2)  # [128,128,2,65]
            Y2 = SHR[:, 0:2 * Wc * BLK].rearrange(
                "p (r a c) -> p r a c", r=2, a=Wc)                 # [128,2,65,96]
            nc.sync.dma_start(out=Xri[96:97, :, :, :],
                              in_=on_ext[:, 0:H * 130])
            O1 = spool.tile([97, 2, 3, CHK], BF16, tag="o1")      # 3-chunk ring
            nc.sync.dma_start(out=O1[96:97, :, :, :],
                              in_=on_ext[:, 0:2 * 3 * CHK])
            # wc-major so P1's DMA-transpose input [96, hk] is contiguous
            O2 = spool.tile([96, 2, Wc, H], BF16, tag="o2")
            # iH out [h, ri, c, wcpad]: wcpad=128 for P2 transpose; pad cols
            # 65:128 are never read downstream (IW reads Zp partitions 0:65).
            Z = spool.tile([128, 2 * BLK * 128], BF16, tag="z")
            Z3 = Z.rearrange("p (r c a) -> p r c a", r=2, c=BLK)   # [128,2,96,128]
            Zp = spool.tile([128, 2, 2, 12, 128], BF16, tag="zp")  # 2 c-eighths
            Zpf = Zp.rearrange("p s r a b -> p s r (a b)")

            # alternate PSUM-drain engines (Pool cannot read PSUM)
            rr = [0]

            def drain(out, in_):
                rr[0] += 1
                if rr[0] % 2:
                    nc.vector.tensor_copy(out, in_)
                else:
                    nc.scalar.copy(out, in_)

            Zf = Z3.rearrange("p r c a -> p r (c a)")
            O2f = O2.rearrange("p r a b -> p r (a b)")

            def emit_load(b):
                nc.gpsimd.dma_start(out=X32f, in_=x_ext[b])

            def emit_s1_group(c0):
                p1 = psC.tile([128, 390], F32, tag="psC")
                for j in range(3):
                    nc.tensor.matmul(p1[:, j * 130:(j + 1) * 130],
                                     X32[:, :, c0 + j], fh2[:],
                                     start=True, stop=True)
                drain(ZtT[:, :, c0:c0 + 3],
                      p1.rearrange("p (c k) -> p k c", c=3))

            def emit_p1():
                # in [96, (wc.128hk)] -> out [128hk, wc, 96c]
                nc.sync.dma_start(out=Y2[:, 0], in_=O2f[:, 0], transpose=True)
                nc.sync.dma_start(out=Y2[:, 1], in_=O2f[:, 1], transpose=True)

            def emit_ih_chunk(w0):
                wn = 5
                n = wn * BLK
                yr = Y2[:, 0, w0:w0 + wn, :]
                yi = Y2[:, 1, w0:w0 + wn, :]
                pzr = psA.tile([128, 480], F32, tag="psA")
                pzi = psA.tile([128, 480], F32, tag="psA")
                # same stationary (Chi) back-to-back across the two psums
                nc.tensor.matmul(pzr[:, :n], fhi3[:, 0:128], yr,
                                 start=True, stop=False)
                nc.tensor.matmul(pzi[:, :n], fhi3[:, 0:128], yi,
                                 start=True, stop=False)
                nc.tensor.matmul(pzr[:, :n], fhi3[:, 128:256], yi,
                                 start=False, stop=True)
                nc.tensor.matmul(pzi[:, :n], fhi3[:, 256:384], yr,
                                 start=False, stop=True)
                drain(Z3[:, 0, :, w0:w0 + wn],
                      pzr[:, :n].rearrange("p (a b) -> p b a", a=wn))
                drain(Z3[:, 1, :, w0:w0 + wn],
                      pzi[:, :n].rearrange("p (a b) -> p b a", a=wn))

            def emit_ones():
                # restore the ones row (P1 overwrote partition 96 of SHR)
                nc.sync.dma_start(out=Xri[96:97, :, :, :],
                                  in_=on_ext[:, 0:H * 130])

            def emit_p2(e):
                s = e % 2
                nc.sync.dma_start(out=Zp[:, s, 0],
                                  in_=Zf[:, 0, e * 1536:(e + 1) * 1536],
                                  transpose=True)
                nc.sync.dma_start(out=Zp[:, s, 1],
                                  in_=Zf[:, 1, e * 1536:(e + 1) * 1536],
                                  transpose=True)

            def emit_iw(b, e):
                s = e % 2
                # 3 chunks of 512; first two share stationary loads
                p5a = psA.tile([128, 512], F32, tag="psA")
                p5b = psA.tile([128, 512], F32, tag="psA")
                sla, slb = slice(0, 512), slice(512, 1024)
                nc.tensor.matmul(p5a[:], fwi2[:, 0:128],
                                 Zpf[0:65, s, 0, sla], start=True, stop=False)
                nc.tensor.matmul(p5b[:], fwi2[:, 0:128],
                                 Zpf[0:65, s, 0, slb], start=True, stop=False)
                nc.tensor.matmul(p5a[:], fwi2[:, 128:256],
                                 Zpf[0:65, s, 1, sla], start=False, stop=True)
                nc.tensor.matmul(p5b[:], fwi2[:, 128:256],
                                 Zpf[0:65, s, 1, slb], start=False, stop=True)
                for p5, kk in ((p5a, 0), (p5b, 1)):
                    ot = stg2.tile([128, 512], BF16, tag="ot")
                    drain(ot, p5[:])
                    nc.gpsimd.dma_start(out=out_ext[b, e * 3 + kk], in_=ot)
                p5c = psA.tile([128, 512], F32, tag="psA")
                slc = slice(1024, 1536)
                nc.tensor.matmul(p5c[:], fwi2[:, 0:128],
                                 Zpf[0:65, s, 0, slc], start=True, stop=False)
                nc.tensor.matmul(p5c[:], fwi2[:, 128:256],
                                 Zpf[0:65, s, 1, slc], start=False, stop=True)
                ot = stg2.tile([128, 512], BF16, tag="ot")
                drain(ot, p5c[:])
                nc.gpsimd.dma_start(out=out_ext[b, e * 3 + 2], in_=ot)

            kchunk = [0]

            def emit_l_chunk(j):
                h0 = 7 * j
                hn = min(7, H - h0)
                n = hn * Wc
                xr = Xri[0:97, h0:h0 + hn, 0, :]
                xi = Xri[0:97, h0:h0 + hn, 1, :]
                pr = psB.tile([96, CHK], F32, tag="psB")
                pi = psB.tile([96, CHK], F32, tag="psB")
                nc.tensor.matmul(pr[:, :n], W1ra, xr, start=True, stop=False)
                nc.tensor.matmul(pr[:, :n], W1mi, xi[0:96],
                                 start=False, stop=True)
                nc.tensor.matmul(pi[:, :n], W1ib, xr, start=True, stop=False)
                nc.tensor.matmul(pi[:, :n], W1rb, xi[0:96],
                                 start=False, stop=True)
                kr = kchunk[0] % 3
                kchunk[0] += 1
                nc.vector.tensor_scalar(O1[0:96, 0, kr, :n], pr[:, :n],
                                        0.0, None, MAX)
                nc.scalar.activation(O1[0:96, 1, kr, :n], pi[:, :n], RELU)

                # L2 on the chunk just produced
                o1r = O1[:, 0, kr, :n]
                o1i = O1[:, 1, kr, :n]
                qr = psB.tile([96, CHK], F32, tag="psB")
                qi = psB.tile([96, CHK], F32, tag="psB")
                nc.tensor.matmul(qr[:, :n], W2ra, o1r, start=True, stop=False)
                nc.tensor.matmul(qr[:, :n], W2mi, o1i[0:96],
                                 start=False, stop=True)
                nc.tensor.matmul(qi[:, :n], W2ib, o1r, start=True, stop=False)
                nc.tensor.matmul(qi[:, :n], W2rb, o1i[0:96],
                                 start=False, stop=True)
                # O2 dst written contiguously (wc-major); psum operands
                # read with (wc, hk)-permuted APs instead
                o2r = O2[:, 0, :, h0:h0 + hn]
                o2i = O2[:, 1, :, h0:h0 + hn]
                qrv = qr[:, :n].rearrange("p (a b) -> p b a", b=Wc)
                t1 = stg.tile([96, CHK], F32, tag="clip")
                t1v = t1[:, :n].rearrange("p (a b) -> p b a", b=Wc)
                # real: softshrink = y - clip(y) on DVE
                nc.vector.tensor_scalar(t1[:, :n], qr[:, :n], LAM, -LAM,
                                        MIN, MAX)
                nc.vector.tensor_tensor(o2r, qrv, t1v, SUB)
                # imag: relu(y-lam) - relu(-y-lam) on Act, sub on Pool
                sa = stg.tile([96, CHK], BF16, tag="sa")
                sb = stg.tile([96, CHK], BF16, tag="sb")
                sav = sa[:, :n].rearrange("p (a b) -> p b a", b=Wc)
                sbv = sb[:, :n].rearrange("p (a b) -> p b a", b=Wc)
                nc.scalar.activation(sa[:, :n], qi[:, :n], RELU, bias=mlam)
                nc.scalar.activation(sb[:, :n], qi[:, :n], RELU, bias=mlam,
                                     scale=-1.0)
                nc.gpsimd.tensor_tensor(o2i, sav, sbv, SUB)

            def emit_s2_pair(k):
                ps = psC.tile([96, 260], F32, tag="psC")
                nc.tensor.matmul(ps[:], ZtT[:, k, :], fw4[:],
                                 start=True, stop=False)
                nc.tensor.matmul(ps[:], ZtT[:, 65 + k, :], fw4i[:],
                                 start=False, stop=True)
                if 1 <= k <= 63:
                    # one drain for rows k and 128-k via step-slice
                    dst = Xri[0:96, k:129 - k:128 - 2 * k, :, :]
                    drain(dst, ps.rearrange("p (g r a) -> p g r a", g=2, r=2))
                else:
                    drain(Xri[0:96, k, :, :], ps[:, 0:130])

            # ---- software-pipelined schedule: sample b's inverse phase is
            # ---- emitted interleaved into sample b+1's forward phase so the
            # ---- in-order PE stream has fill work during DMA transposes.
            emit_load(0)
            for b in range(B):
                if b > 0:
                    emit_p1()  # P1(b-1); Y2 reuses Xri(b-1) buffer
                ih_sched = {16 + i: w0 for i, w0 in
                            enumerate(range(0, Wc, 5))}  # groups 16..28
                for g, c0 in enumerate(range(0, BLK, 3)):
                    emit_s1_group(c0)
                    if b > 0 and g in ih_sched:
                        emit_ih_chunk(ih_sched[g])
                if b > 0:
                    emit_ones()
                for k in range(65):
                    emit_s2_pair(k)
                    if k == 2 and b + 1 < B:
                        emit_load(b + 1)
                    if b > 0 and k % 8 == 0 and k // 8 < 8:
                        emit_p2(k // 8)
                    if b > 0 and k % 8 == 4:
                        emit_iw(b - 1, (k - 4) // 8)
                    for j in ready.get(k, []):
                        emit_l_chunk(j)

            # tail: inverse phase of the last sample
            emit_p1()
            for w0 in range(0, Wc, 5):
                emit_ih_chunk(w0)
            for e in range(8):
                emit_p2(e)
                emit_iw(B - 1, e)

    nc.compile()
    return nc


def kernel(x, w1, b1, w2, b2):
    x = np.ascontiguousarray(x, dtype=np.float32)
    key = "nc"
    if key not in _cache:
        _cache[key] = _build_graph()
    nc = _cache[key]

    in_maps = make_in_maps(x, w1, b1, w2, b2)
    res = run_bass_kernel_spmd(nc, in_maps, core_ids=list(range(NCORES)))
    # device layout [B, 24, w, 512] -> [B, w, c, h] -> [B, h, w, c]
    parts = []
    for i in range(NCORES):
        r = np.asarray(res.results[i]["out"], dtype=np.float32)
        r = r.reshape(B, 24, W, 4, H).transpose(0, 4, 2, 1, 3)
        parts.append(r.reshape(B, H, W, BLK))
    corr = np.concatenate(parts, axis=3)
    return (corr + x).astype(np.float32)


def make_in_maps(x, w1, b1, w2, b2):
    fh2, fw4, fw4i, fwi2, fhi3, ones = _build_consts()
    in_maps = []
    for i in range(NCORES):
        w1p, w2p = _pack_mlp(w1, b1, w2, b2, i)
        in_maps.append({
            "x": np.ascontiguousarray(x[:, :, :, i * BLK:(i + 1) * BLK]),
            "fh2": fh2, "fw4": fw4, "fw4i": fw4i, "fwi2": fwi2, "fhi3": fhi3,
            "w1p": w1p, "w2p": w2p, "ones": ones,
            "mlam": np.full((96, 1), -LAM, dtype=np.float32),
        })
    return in_maps


# revision 12
# speedup vs baseline: 1.5222x; 1.0086x over previous
"""AFNO2D layer on 8 TRN2 NeuronCores.

Sharding: channel-block parallel. Core i owns channels [96*i, 96*(i+1)) —
exactly block i of the block-diagonal MLP. No collectives.

v3: Hermitian forward DFT, contiguous-stationary S2 with paired single
drains, IH with contiguous moving operand, and cross-sample overlap
(sample b+1's S1/S2 fills the PE idle while sample b runs P1/IH/P2/IW).

Per core, per batch sample (tokens t = hk*65 + wc, NT = 8320):
  S1  H-DFT, kh=0..64 only (real input => Hermitian in kh).
      lhsT=x_c [h,w], rhs=fh2=[Ch|Sh] [128,130] -> psum [w, 130] per ch.
      Drain transposes into ZtT [w, khri(130), c] (strided DVE/ACT write)
      so S2's stationary loads are contiguous. Own PSUM pool (psC) so it
      can run while the previous sample's inverse phase occupies psA.
  S2  W-rDFT per kh-pair (k, 128-k): rows share the products Ztr@{Cw,Sw},
      Zti@{Sw,Cw}: 2 contiguous LDW + 2 MM N=260 per pair. One drain per
      pair via a step-sliced Xri view covering rows k and 128-k.
  L1  MLP layer 1 (bias via ones-row), relu drain; chunks of 7 hk,
      emitted as soon as their S2 pairs are done.
  L2  MLP layer 2, softshrink drain -> O2 [c, ri, wc, hk].
  P1  DMA transpose -> Y2 [hk, ri, wc, c]. Y2 lives inside the Xri
      buffer (Xri is fully consumed before P1 writes); the ones-row
      (partition 96) is re-DMAed after IH reads.
  IH  H-iDFT F-stationary, moving = wc-chunks of Y2 (contiguous),
      strided drain -> Z [h, ri, c, wcpad(128)].
  P2  DMA transpose c-eighths -> Zp [wcpad, ri, c12, h] (double-buffered)
  IW  W-irDFT F-stationary: lhsT=fwi2=[Cwi|-Swi] -> [w, 512]-chunks -> HBM
Residual add + final transpose run on the host in fp32.
"""
import sys
import types
import numpy as np
import ml_dtypes

# run_bass_kernel_spmd(trace=True) needs this hook module; missing in image.
if "antenv.axon_hooks" not in sys.modules:
    _hooks_mod = types.ModuleType("antenv.axon_hooks")
    _hooks_mod._hook = None
    _hooks_mod.set_axon_ntff_profile_hook = lambda h: setattr(_hooks_mod, "_hook", h)
    _hooks_mod.get_axon_ntff_profile_hook = lambda: _hooks_mod._hook
    sys.modules["antenv.axon_hooks"] = _hooks_mod
    try:
        sys.path.insert(0, "/root/.axon_site")
        from trn_agent_boot.trn_boot import _ntff_profile_via_ctypes
        _hooks_mod._hook = _ntff_profile_via_ctypes("/opt/axon/libaxon_pjrt.so")
    except Exception:
        pass

import concourse.bacc as bacc
import concourse.tile as tile
from concourse import mybir
from concourse.bass_utils import run_bass_kernel_spmd

F32 = mybir.dt.float32
BF16 = mybir.dt.bfloat16

B, H, W, C = 4, 128, 128, 768
Wc = W // 2 + 1            # 65
NCORES, BLK = 8, 96        # channels per core
NT = H * Wc                # 8320 tokens per sample
LAM = 0.01
CHK = 455                  # 7 hk per MLP chunk

_cache = {}


def _build_consts():
    bf = ml_dtypes.bfloat16
    h = np.arange(H)
    k65 = np.arange(Wc)
    wc = np.arange(Wc)
    w = np.arange(W)
    hk = np.arange(H)
    ang_h = 2 * np.pi * np.outer(h, k65) / H
    Ch, Sh = np.cos(ang_h) / np.sqrt(H), -np.sin(ang_h) / np.sqrt(H)
    ang_w = 2 * np.pi * np.outer(w, wc) / W
    Cw, Sw = np.cos(ang_w) / np.sqrt(W), -np.sin(ang_w) / np.sqrt(W)
    alpha = np.ones(Wc)
    alpha[1:64] = 2.0
    ang_wi = 2 * np.pi * np.outer(wc, w) / W
    Cwi = alpha[:, None] * np.cos(ang_wi) / np.sqrt(W)
    Swi = alpha[:, None] * np.sin(ang_wi) / np.sqrt(W)
    ang_hi = 2 * np.pi * np.outer(hk, h) / H
    Chi, Shi = np.cos(ang_hi) / np.sqrt(H), np.sin(ang_hi) / np.sqrt(H)

    fh2 = np.concatenate([Ch, Sh], axis=1).astype(bf)                  # [128,130]
    # S2 pair trick: psum = Ztr@fw4 + Zti@fw4i
    #   cols 0:130   -> row k      (r|i)
    #   cols 130:260 -> row 128-k  (r|i)
    fw4 = np.concatenate([Cw, Sw, Cw, Sw], axis=1).astype(bf)          # [128,260]
    fw4i = np.concatenate([-Sw, Cw, Sw, -Cw], axis=1).astype(bf)       # [128,260]
    # iW (final, real out): out = Cwi^T Zpr + (-Swi)^T Zpi
    fwi2 = np.concatenate([Cwi, -Swi], axis=1).astype(bf)              # [65,256]
    # iH (complex): Zr = Chi^T Yr - Shi^T Yi ; Zi = Shi^T Yr + Chi^T Yi
    fhi3 = np.concatenate([Chi, -Shi, Shi], axis=1).astype(bf)         # [128,384]
    ones = np.ones((1, 2 * NT), dtype=np.float32).astype(bf)           # [1,16640]
    return fh2, fw4, fw4i, fwi2, fhi3, ones


def _pack_mlp(w1, b1, w2, b2, blk):
    """[97, 384] packs: cols [Wr;br | -Wi;0 | Wi;bi | Wr;0]."""
    def pack(wr, wi, br, bi):
        p = np.zeros((97, 384), dtype=np.float32)
        p[:96, 0:96] = wr
        p[96, 0:96] = br
        p[:96, 96:192] = -wi
        p[:96, 192:288] = wi
        p[96, 192:288] = bi
        p[:96, 288:384] = wr
        return p
    w1p = pack(w1[0, blk], w1[1, blk], b1[0, blk], b1[1, blk])
    w2p = pack(w2[0, blk], w2[1, blk], b2[0, blk], b2[1, blk])
    return w1p, w2p


def _build_graph():
    nc = bacc.Bacc("TRN2", target_bir_lowering=False, debug=False,
                   num_devices=NCORES)

    # x is pre-transposed on the host to [B, H, C, W] so S1's stationary
    # loads (per-channel [h, w] slices) are contiguous -> fast weight load.
    x_ext = nc.dram_tensor("x", [B, H, BLK, W], F32, kind="ExternalInput").ap()
    fh_ext = nc.dram_tensor("fh2", [128, 130], BF16, kind="ExternalInput").ap()
    fw_ext = nc.dram_tensor("fw4", [128, 260], BF16, kind="ExternalInput").ap()
    fwi_ext2 = nc.dram_tensor("fw4i", [128, 260], BF16, kind="ExternalInput").ap()
    fwi_ext = nc.dram_tensor("fwi2", [65, 256], BF16, kind="ExternalInput").ap()
    fhi_ext = nc.dram_tensor("fhi3", [128, 384], BF16, kind="ExternalInput").ap()
    w1_ext = nc.dram_tensor("w1p", [97, 384], F32, kind="ExternalInput").ap()
    w2_ext = nc.dram_tensor("w2p", [97, 384], F32, kind="ExternalInput").ap()
    on_ext = nc.dram_tensor("ones", [1, 2 * NT], BF16, kind="ExternalInput").ap()
    ml_ext = nc.dram_tensor("mlam", [96, 1], F32, kind="ExternalInput").ap()
    # device out: 24 chunks of [w, 512] over (c h)-flat; host reassembles
    out_ext = nc.dram_tensor("out", [B, 24, W, 512], BF16,
                             kind="ExternalOutput").ap()

    SUB = mybir.AluOpType.subtract
    MIN = mybir.AluOpType.min
    MAX = mybir.AluOpType.max
    RELU = mybir.ActivationFunctionType.Relu

    # L1/L2 chunk j is ready after S2 pair max-unit mu(j)
    ready = {}
    for j in range(19):
        rows = range(7 * j, min(7 * j + 7, H))
        mu = max(r if r <= 64 else H - r for r in rows)
        ready.setdefault(mu, []).append(j)

    with tile.TileContext(nc) as tc:
        with (
            tc.tile_pool(name="consts", bufs=1) as cpool,
            tc.tile_pool(name="stat", bufs=1) as spool,
            tc.tile_pool(name="stg", bufs=2) as stg,      # clip staging
            tc.tile_pool(name="stg2", bufs=3) as stg2,    # out staging
            tc.tile_pool(name="psA", bufs=3, space="PSUM") as psA,  # IH/IW
            tc.tile_pool(name="psB", bufs=3, space="PSUM") as psB,  # S2/L1/L2
            tc.tile_pool(name="psC", bufs=2, space="PSUM") as psC,  # S1
        ):
            # ---- constants / weights to SBUF (once) ----
            fh2 = cpool.tile([128, 130], BF16, tag="fh2")
            nc.sync.dma_start(out=fh2, in_=fh_ext)
            fw4 = cpool.tile([128, 260], BF16, tag="fw4")
            nc.sync.dma_start(out=fw4, in_=fw_ext)
            fw4i = cpool.tile([128, 260], BF16, tag="fw4i")
            nc.sync.dma_start(out=fw4i, in_=fwi_ext2)
            fwi2 = cpool.tile([65, 256], BF16, tag="fwi2")
            nc.sync.dma_start(out=fwi2, in_=fwi_ext)
            fhi3 = cpool.tile([128, 384], BF16, tag="fhi3")
            nc.sync.dma_start(out=fhi3, in_=fhi_ext)
            w1p = cpool.tile([97, 384], BF16, tag="w1p")
            nc.gpsimd.dma_start(out=w1p, in_=w1_ext)      # casting DMA
            w2p = cpool.tile([97, 384], BF16, tag="w2p")
            nc.gpsimd.dma_start(out=w2p, in_=w2_ext)
            mlam = cpool.tile([96, 1], F32, tag="mlam")
            nc.sync.dma_start(out=mlam, in_=ml_ext)

            # W slices: lhsT [K, 96]
            W1ra = w1p[:, 0:96]          # [97, 96] row96 = b1r
            W1mi = w1p[0:96, 96:192]     # -Wi
            W1ib = w1p[:, 192:288]       # Wi ; b1i
            W1rb = w1p[0:96, 288:384]    # Wr
            W2ra = w2p[:, 0:96]
            W2mi = w2p[0:96, 96:192]
            W2ib = w2p[:, 192:288]
            W2rb = w2p[0:96, 288:384]

            # ---- static tiles (time-shared across stages/samples) ----
            X32f = spool.tile([128, W * BLK], BF16, tag="x32")
            X32 = X32f.rearrange("p (c w) -> p c w", c=BLK)
            ZtTf = spool.tile([128, 130 * BLK], BF16, tag="ztt")
            ZtT = ZtTf.rearrange("p (k c) -> p k c", c=BLK)        # [128,130,96]
            # Xri [97(+ones row), hk, ri, wc] shares its buffer with Y2
            # (P1 out, [hk, ri, wc, c]): L1 fully consumes Xri before P1
            # writes; the ones row (partition 96) is re-DMAed after IH.
            SHR = spool.tile([128, H * 130], BF16, tag="shr")
            Xri = SHR.rearrange("p (h r a) -> p h r a", h=H, r=2)  # [128,128,2,65]
            Y2 = SHR[:, 0:2 * Wc * BLK].rearrange(
                "p (r a c) -> p r a c", r=2, a=Wc)                 # [128,2,65,96]
            nc.sync.dma_start(out=Xri[96:97, :, :, :],
                              in_=on_ext[:, 0:H * 130])
            O1 = spool.tile([97, 2, 3, CHK], BF16, tag="o1")      # 3-chunk ring
            nc.sync.dma_start(out=O1[96:97, :, :, :],
                              in_=on_ext[:, 0:2 * 3 * CHK])
            # wc-major so P1's DMA-transpose input [96, hk] is contiguous
            O2 = spool.tile([96, 2, Wc, H], BF16, tag="o2")
            # iH out [h, ri, c, wcpad]: wcpad=128 for P2 transpose; pad cols
            # 65:128 are never read downstream (IW reads Zp partitions 0:65).
            Z = spool.tile([128, 2 * BLK * 128], BF16, tag="z")
            Z3 = Z.rearrange("p (r c a) -> p r c a", r=2, c=BLK)   # [128,2,96,128]
            Zp = spool.tile([128, 2, 2, 12, 128], BF16, tag="zp")  # 2 c-eighths
            Zpf = Zp.rearrange("p s r a b -> p s r (a b)")

            # alternate PSUM-drain engines (Pool cannot read PSUM)
            rr = [0]

            def drain(out, in_):
                rr[0] += 1
                if rr[0] % 2:
                    nc.vector.tensor_copy(out, in_)
                else:
                    nc.scalar.copy(out, in_)

            Zf = Z3.rearrange("p r c a -> p r (c a)")
            O2f = O2.rearrange("p r a b -> p r (a b)")

            def emit_load(b):
                nc.gpsimd.dma_start(out=X32f, in_=x_ext[b])

            def emit_s1_group(c0):
                p1 = psC.tile([128, 390], F32, tag="psC")
                for j in range(3):
                    nc.tensor.matmul(p1[:, j * 130:(j + 1) * 130],
                                     X32[:, c0 + j, :], fh2[:],
                                     start=True, stop=True)
                drain(ZtT[:, :, c0:c0 + 3],
                      p1.rearrange("p (c k) -> p k c", c=3))

            def emit_p1(half):
                # in [96, (wc.128hk)] -> out [128hk, wc, 96c]; split by wc
                # halves (block-diagonal per wc) so IH can start sooner
                lo, hi = (0, 35) if half == 0 else (35, Wc)
                sl = slice(lo * H, hi * H)
                nc.sync.dma_start(out=Y2[:, 0, lo:hi, :], in_=O2f[:, 0, sl],
                                  transpose=True)
                nc.sync.dma_start(out=Y2[:, 1, lo:hi, :], in_=O2f[:, 1, sl],
                                  transpose=True)

            def emit_ih_chunk(w0):
                wn = 5
                n = wn * BLK
                yr = Y2[:, 0, w0:w0 + wn, :]
                yi = Y2[:, 1, w0:w0 + wn, :]
                pzr = psA.tile([128, 480], F32, tag="psA")
                pzi = psA.tile([128, 480], F32, tag="psA")
                # same stationary (Chi) back-to-back across the two psums
                nc.tensor.matmul(pzr[:, :n], fhi3[:, 0:128], yr,
                                 start=True, stop=False)
                nc.tensor.matmul(pzi[:, :n], fhi3[:, 0:128], yi,
                                 start=True, stop=False)
                nc.tensor.matmul(pzr[:, :n], fhi3[:, 128:256], yi,
                                 start=False, stop=True)
                nc.tensor.matmul(pzi[:, :n], fhi3[:, 256:384], yr,
                                 start=False, stop=True)
                drain(Z3[:, 0, :, w0:w0 + wn],
                      pzr[:, :n].rearrange("p (a b) -> p b a", a=wn))
                drain(Z3[:, 1, :, w0:w0 + wn],
                      pzi[:, :n].rearrange("p (a b) -> p b a", a=wn))

            def emit_ones():
                # restore the ones row (P1 overwrote partition 96 of SHR)
                nc.sync.dma_start(out=Xri[96:97, :, :, :],
                                  in_=on_ext[:, 0:H * 130])

            def emit_p2(e):
                s = e % 2
                nc.sync.dma_start(out=Zp[:, s, 0],
                                  in_=Zf[:, 0, e * 1536:(e + 1) * 1536],
                                  transpose=True)
                nc.sync.dma_start(out=Zp[:, s, 1],
                                  in_=Zf[:, 1, e * 1536:(e + 1) * 1536],
                                  transpose=True)

            def emit_iw(b, e):
                s = e % 2
                # 3 chunks of 512; first two share stationary loads
                p5a = psA.tile([128, 512], F32, tag="psA")
                p5b = psA.tile([128, 512], F32, tag="psA")
                sla, slb = slice(0, 512), slice(512, 1024)
                nc.tensor.matmul(p5a[:], fwi2[:, 0:128],
                                 Zpf[0:65, s, 0, sla], start=True, stop=False)
                nc.tensor.matmul(p5b[:], fwi2[:, 0:128],
                                 Zpf[0:65, s, 0, slb], start=True, stop=False)
                nc.tensor.matmul(p5a[:], fwi2[:, 128:256],
                                 Zpf[0:65, s, 1, sla], start=False, stop=True)
                nc.tensor.matmul(p5b[:], fwi2[:, 128:256],
                                 Zpf[0:65, s, 1, slb], start=False, stop=True)
                for p5, kk in ((p5a, 0), (p5b, 1)):
                    ot = stg2.tile([128, 512], BF16, tag="ot")
                    drain(ot, p5[:])
                    nc.gpsimd.dma_start(out=out_ext[b, e * 3 + kk], in_=ot)
                p5c = psA.tile([128, 512], F32, tag="psA")
                slc = slice(1024, 1536)
                nc.tensor.matmul(p5c[:], fwi2[:, 0:128],
                                 Zpf[0:65, s, 0, slc], start=True, stop=False)
                nc.tensor.matmul(p5c[:], fwi2[:, 128:256],
                                 Zpf[0:65, s, 1, slc], start=False, stop=True)
                ot = stg2.tile([128, 512], BF16, tag="ot")
                drain(ot, p5c[:])
                nc.gpsimd.dma_start(out=out_ext[b, e * 3 + 2], in_=ot)

            kchunk = [0]

            def emit_l_chunk(j):
                h0 = 7 * j
                hn = min(7, H - h0)
                n = hn * Wc
                xr = Xri[0:97, h0:h0 + hn, 0, :]
                xi = Xri[0:97, h0:h0 + hn, 1, :]
                pr = psB.tile([96, CHK], F32, tag="psB")
                pi = psB.tile([96, CHK], F32, tag="psB")
                nc.tensor.matmul(pr[:, :n], W1ra, xr, start=True, stop=False)
                nc.tensor.matmul(pr[:, :n], W1mi, xi[0:96],
                                 start=False, stop=True)
                nc.tensor.matmul(pi[:, :n], W1ib, xr, start=True, stop=False)
                nc.tensor.matmul(pi[:, :n], W1rb, xi[0:96],
                                 start=False, stop=True)
                kr = kchunk[0] % 3
                kchunk[0] += 1
                nc.vector.tensor_scalar(O1[0:96, 0, kr, :n], pr[:, :n],
                                        0.0, None, MAX)
                nc.scalar.activation(O1[0:96, 1, kr, :n], pi[:, :n], RELU)

                # L2 on the chunk just produced
                o1r = O1[:, 0, kr, :n]
                o1i = O1[:, 1, kr, :n]
                qr = psB.tile([96, CHK], F32, tag="psB")
                qi = psB.tile([96, CHK], F32, tag="psB")
                nc.tensor.matmul(qr[:, :n], W2ra, o1r, start=True, stop=False)
                nc.tensor.matmul(qr[:, :n], W2mi, o1i[0:96],
                                 start=False, stop=True)
                nc.tensor.matmul(qi[:, :n], W2ib, o1r, start=True, stop=False)
                nc.tensor.matmul(qi[:, :n], W2rb, o1i[0:96],
                                 start=False, stop=True)
                # O2 dst written contiguously (wc-major); psum operands
                # read with (wc, hk)-permuted APs instead
                o2r = O2[:, 0, :, h0:h0 + hn]
                o2i = O2[:, 1, :, h0:h0 + hn]
                qrv = qr[:, :n].rearrange("p (a b) -> p b a", b=Wc)
                t1 = stg.tile([96, CHK], F32, tag="clip")
                t1v = t1[:, :n].rearrange("p (a b) -> p b a", b=Wc)
                # real: softshrink = y - clip(y) on DVE
                nc.vector.tensor_scalar(t1[:, :n], qr[:, :n], LAM, -LAM,
                                        MIN, MAX)
                nc.vector.tensor_tensor(o2r, qrv, t1v, SUB)
                # imag: relu(y-lam) - relu(-y-lam) on Act, sub on Pool
                sa = stg.tile([96, CHK], BF16, tag="sa")
                sb = stg.tile([96, CHK], BF16, tag="sb")
                sav = sa[:, :n].rearrange("p (a b) -> p b a", b=Wc)
                sbv = sb[:, :n].rearrange("p (a b) -> p b a", b=Wc)
                nc.scalar.activation(sa[:, :n], qi[:, :n], RELU, bias=mlam)
                nc.scalar.activation(sb[:, :n], qi[:, :n], RELU, bias=mlam,
                                     scale=-1.0)
                nc.gpsimd.tensor_tensor(o2i, sav, sbv, SUB)

            def emit_s2_pair(k):
                ps = psC.tile([96, 260], F32, tag="psC")
                nc.tensor.matmul(ps[:], ZtT[:, k, :], fw4[:],
                                 start=True, stop=False)
                nc.tensor.matmul(ps[:], ZtT[:, 65 + k, :], fw4i[:],
                                 start=False, stop=True)
                if 1 <= k <= 63:
                    # one drain for rows k and 128-k via step-slice
                    dst = Xri[0:96, k:129 - k:128 - 2 * k, :, :]
                    drain(dst, ps.rearrange("p (g r a) -> p g r a", g=2, r=2))
                else:
                    drain(Xri[0:96, k, :, :], ps[:, 0:130])

            # ---- software-pipelined schedule: sample b's inverse phase is
            # ---- emitted interleaved into sample b+1's forward phase so the
            # ---- in-order PE stream has fill work during DMA transposes.
            emit_load(0)
            # P2(e) issue pair -> list of eighths; IW(e) runs at pair 8e+8
            p2_at = {0: [0, 1]}
            for e in range(2, 8):
                p2_at[8 * (e - 1)] = [e]
            for b in range(B):
                if b > 0:
                    emit_p1(0)  # P1(b-1); Y2 reuses Xri(b-1) buffer
                ih_sched = {16 + i: w0 for i, w0 in
                            enumerate(range(0, Wc, 5))}  # groups 16..28
                for g, c0 in enumerate(range(0, BLK, 3)):
                    emit_s1_group(c0)
                    if b > 0 and g == 6:
                        emit_p1(1)
                    if b > 0 and g in ih_sched:
                        emit_ih_chunk(ih_sched[g])
                if b > 0:
                    emit_ones()
                for k in range(65):
                    emit_s2_pair(k)
                    if k == 2 and b + 1 < B:
                        emit_load(b + 1)
                    if b > 0 and k % 8 == 0 and 1 <= k // 8 <= 8:
                        emit_iw(b - 1, k // 8 - 1)
                    if b > 0 and k in p2_at:
                        for e in p2_at[k]:
                            emit_p2(e)
                    for j in ready.get(k, []):
                        emit_l_chunk(j)

            # tail: inverse phase of the last sample
            emit_p1(0)
            emit_p1(1)
            for w0 in range(0, Wc, 5):
                emit_ih_chunk(w0)
            emit_p2(0)
            emit_p2(1)
            for e in range(8):
                emit_iw(B - 1, e)
                if e + 2 < 8:
                    emit_p2(e + 2)

    nc.compile()
    return nc


def kernel(x, w1, b1, w2, b2):
    x = np.ascontiguousarray(x, dtype=np.float32)
    key = "nc"
    if key not in _cache:
        _cache[key] = _build_graph()
    nc = _cache[key]

    in_maps = make_in_maps(x, w1, b1, w2, b2)
    res = run_bass_kernel_spmd(nc, in_maps, core_ids=list(range(NCORES)))
    # device layout [B, 24, w, 512] -> [B, w, c, h] -> [B, h, w, c]
    parts = []
    for i in range(NCORES):
        r = np.asarray(res.results[i]["out"], dtype=np.float32)
        r = r.reshape(B, 24, W, 4, H).transpose(0, 4, 2, 1, 3)
        parts.append(r.reshape(B, H, W, BLK))
    corr = np.concatenate(parts, axis=3)
    return (corr + x).astype(np.float32)


def make_in_maps(x, w1, b1, w2, b2):
    fh2, fw4, fw4i, fwi2, fhi3, ones = _build_consts()
    in_maps = []
    for i in range(NCORES):
        w1p, w2p = _pack_mlp(w1, b1, w2, b2, i)
        in_maps.append({
            "x": np.ascontiguousarray(
                x[:, :, :, i * BLK:(i + 1) * BLK].transpose(0, 1, 3, 2)),
            "fh2": fh2, "fw4": fw4, "fw4i": fw4i, "fwi2": fwi2, "fhi3": fhi3,
            "w1p": w1p, "w2p": w2p, "ones": ones,
            "mlam": np.full((96, 1), -LAM, dtype=np.float32),
        })
    return in_maps
